# revision 27
# baseline (speedup 1.0000x reference)
"""Trainium2 Bass kernel for nn_EncoderLayer (B=4, S=1024, D=1024, H=16, FF=2048).

Sharding: 8 cores = 4 batches x 2 sequence-halves, each core redundantly
computes K/V for its whole batch and runs the layer for its own 512 query
rows. Odd cores see the sequence rotated by 512 (softmax over keys is
permutation-invariant).

Mixed-precision PE pipeline:
  - K/V projections, QK^T scores, exp, attn*V and the output projection run
    in fp8e4 (TRN E4M3) with MatmulPerfMode.DoubleRow (two 128-row
    contraction tiles per instruction = 2x PE throughput). fp8 weights are
    pre-scaled by 32 on the host; the 1/32 is folded into PSUM->SBUF copies.
  - Q projection and both FFN matmuls stay bf16: their error lands on the
    residual stream where fp8 blows the 2e-2 budget (measured 5e-2 all-fp8
    vs 3.2e-3 with these three in bf16).

Schedule: the softmax exp stream (~57us on the scalar engine) is the
second-longest pole after the PE, so scores for head-pair j are emitted
immediately after the K projection of tile j, with attn*V lagging one pair
(PV(j-1) fills the PE while the vector engine re-quantizes k8[j]). V(c=0)
is interleaved into the j=0 block, V(c=1) into the j=4 block (attn*V for
pairs 0-3 only reads heads 0-7 = the c=0 half of V).

Layernorm plumbing (feature-major activations, stats via ones-column
matmuls, gamma/beta folded into W1 + rank-1 W2 corrections) as in v1; the
LN column-sum matmuls are interleaved (lag-one) into the Wo/FFN2 loops and
the LN2 normalize reads the A/B broadcast PSUM directly and writes bf16.
"""

import sys
import types

import numpy as np
import ml_dtypes


def _shim_axon_hooks():
    try:
        import antenv.axon_hooks  # noqa: F401
    except Exception:
        mod = types.ModuleType("antenv.axon_hooks")
        mod.get_axon_ntff_profile_hook = lambda: None
        mod.set_axon_ntff_profile_hook = lambda h: None
        sys.modules["antenv.axon_hooks"] = mod


_shim_axon_hooks()

from concourse import bacc, mybir, tile  # noqa: E402
from concourse import bass_utils  # noqa: E402

F32 = mybir.dt.float32
F32R = mybir.dt.float32r
BF16 = mybir.dt.bfloat16
FP8 = mybir.dt.float8e4
AF = mybir.ActivationFunctionType
DR = mybir.MatmulPerfMode.DoubleRow

B, S, D, H, DH, FF = 4, 1024, 1024, 16, 64, 2048
SQ = 512
P = 128
DT = D // P
FT = FF // P
ST = S // P
NCORES = 8
EPS = 1e-6
SCALE = 1.0 / 32.0
ALPHA = 32.0

_C_BO = 0
_C_B1 = 8
_C_B2 = 24
_C_G1 = 32
_C_BE1 = 40
_C_G2 = 48
_C_BE2 = 56


def _emit(ctx, tc, aps):
    nc = tc.nc
    (xT_ap, x8_ap, wq_ap, wk_ap, wv_ap, wo_ap, w1_ap, w2_ap, consts_ap,
     ones_ap, onesrow_ap, fold_ap, yT_ap) = aps

    acts = ctx.enter_context(tc.tile_pool(name="acts", bufs=1))
    wts = ctx.enter_context(tc.tile_pool(name="wts", bufs=2))
    e2p = ctx.enter_context(tc.tile_pool(name="e2p", bufs=7))
    sc2 = ctx.enter_context(tc.tile_pool(name="sc2", bufs=2))
    sc1 = ctx.enter_context(tc.tile_pool(name="sc1", bufs=1))
    pp = ctx.enter_context(tc.tile_pool(name="pp", bufs=1, space="PSUM"))
    pss = ctx.enter_context(tc.tile_pool(name="pss", bufs=2, space="PSUM"))
    pvp = ctx.enter_context(tc.tile_pool(name="pvp", bufs=2, space="PSUM"))

    def wload(tag, src_ap, shape, dtype, nsplit=2):
        w = wts.tile(shape, dtype, tag=tag, name=tag)
        step = shape[1] // nsplit
        for q in range(nsplit):
            nc.sync.dma_start(w[:, q * step:(q + 1) * step, :],
                              src_ap[:, q * step:(q + 1) * step, :])
        return w

    # ---- input DMA: xt8 + wk0 first (K0 gates the exp stream), then Q's ----
    xt8 = acts.tile([P, DT, S], FP8, tag="xt8", name="xt8")
    for q in range(DT):
        nc.sync.dma_start(xt8[:, q:q + 1, :], x8_ap[:, q:q + 1, :])
    wk0 = wload("wka", wk_ap[:, 0], [P, DT, P], FP8)
    wqa = wload("wqa", wq_ap[:, 0], [P, DT, P], BF16)
    wqb = wload("wqb", wq_ap[:, 1], [P, DT, P], BF16)
    xtb_a = acts.tile([P, 4, SQ], BF16, tag="xtb_a", name="xtb_a")
    nc.sync.dma_start(xtb_a[:, 0:2, :], xT_ap[:, 0:2, 0:SQ])
    nc.sync.dma_start(xtb_a[:, 2:4, :], xT_ap[:, 2:4, 0:SQ])
    xtb_b = acts.tile([P, 4, SQ], BF16, tag="xtb_b", name="xtb_b")
    nc.sync.dma_start(xtb_b[:, 0:2, :], xT_ap[:, 4:6, 0:SQ])
    nc.sync.dma_start(xtb_b[:, 2:4, :], xT_ap[:, 6:8, 0:SQ])
    consts = acts.tile([P, 64], F32, tag="consts", name="consts")
    nc.sync.dma_start(consts[:], consts_ap[:])
    ones_r = acts.tile([P, 1], F32R, tag="ones", name="ones")
    nc.sync.dma_start(ones_r[:], ones_ap[:])
    ones_row = acts.tile([1, P], F32R, tag="ones_row", name="ones_row")
    nc.sync.dma_start(ones_row[:], onesrow_ap[:])
    ones_b = acts.tile([P, 1], BF16, tag="ones_b", name="ones_b")
    nc.vector.memset(ones_b[:], 1.0)
    fold = acts.tile([1, 2 * D], FP8, tag="fold", name="fold")
    nc.sync.dma_start(fold[:], fold_ap[:])

    def xlo(k):
        return xtb_a[:, k, :] if k < 4 else xtb_b[:, k - 4, :]

    # ---- attention machinery ----
    q_res = []
    q8 = acts.tile([P, DT, SQ], FP8, tag="q8", name="q8")
    vr8 = acts.tile([P, ST, H, DH + 1], FP8, tag="vr8", name="vr8")
    nc.vector.memset(vr8[:, :, :, DH:DH + 1], 1.0)
    attn8 = acts.tile([P, DT, SQ], FP8, tag="attn8", name="attn8")
    k8 = [None] * DT
    e2s = {}
    pvs = {}

    def qpair_halves(j0, wa, wb):
        """Returns two emission closures (k 0:4 and k 4:8 + copies)."""
        box = {}

        def half(lo, hi):
            if lo == 0:
                box["ps"] = pp.tile([P, 2, SQ], F32, tag="ps2", name="ps2")
            ps = box["ps"]
            for k in range(lo, hi):
                nc.tensor.matmul(ps[:, 0, :], wa[:, k, :], xlo(k),
                                 start=(k == 0), stop=(k == DT - 1))
                nc.tensor.matmul(ps[:, 1, :], wb[:, k, :], xlo(k),
                                 start=(k == 0), stop=(k == DT - 1))
            if hi == DT:
                for h in range(2):
                    j = j0 + h
                    qr = acts.tile([P, SQ], F32R, tag=f"qres{j}",
                                   name=f"qres{j}")
                    nc.scalar.copy(qr[:], ps[:, h, :])
                    q_res.append(qr)
                    nc.vector.tensor_copy(q8[:, j, :], ps[:, h, :])

        return [lambda: half(0, 4), lambda: half(4, DT)]

    def qpair(j0, wa, wb):
        for f in qpair_halves(j0, wa, wb):
            f()

    def kproj(j, w):
        ps = pp.tile([P, 2, SQ], F32, tag="ps2", name="ps2")
        for k in range(0, DT, 2):
            nc.tensor.matmul(ps[:, 0, :], w[:, k:k + 2, :],
                             xt8[:, k:k + 2, 0:SQ],
                             start=(k == 0), stop=(k == DT - 2), perf_mode=DR)
            nc.tensor.matmul(ps[:, 1, :], w[:, k:k + 2, :],
                             xt8[:, k:k + 2, SQ:S],
                             start=(k == 0), stop=(k == DT - 2), perf_mode=DR)
        kj = acts.tile([P, S], FP8, tag=f"k8{j}", name=f"k8{j}")
        nc.vector.tensor_scalar_mul(
            kj[:].rearrange("p (c q) -> p c q", c=2), ps[:], 1.0 / ALPHA)
        k8[j] = kj

    def v_pair_halves(c, st0, wv):
        """Two emission closures, one per key tile; one psum tile + copy."""
        box = {}

        def half(si):
            if si == 0:
                box["ps"] = pp.tile([P, 2, SQ], F32, tag="ps2", name="ps2")
            ps = box["ps"]
            for k in range(0, DT, 2):
                nc.tensor.matmul(
                    ps[:, si, :],
                    xt8[:, k:k + 2, (st0 + si) * P:(st0 + si + 1) * P],
                    wv[:, k:k + 2, :],
                    start=(k == 0), stop=(k == DT - 2), perf_mode=DR)
            if si == 1:
                nc.scalar.mul(
                    vr8[:, st0:st0 + 2, c * 8:(c + 1) * 8, 0:DH],
                    ps[:].rearrange("p s (h d) -> p s h d", d=DH),
                    1.0 / ALPHA)

        return [lambda: half(0), lambda: half(1)]

    def scx(j, st2):
        """scores + exp for head pair (2j, 2j+1), key tiles 2*st2, 2*st2+1."""
        e2 = e2p.tile([P, 2, 2, SQ], FP8, tag="e2", name="e2")
        for sti in range(2):
            st = 2 * st2 + sti
            sl = slice(st * P, (st + 1) * P)
            ps = pss.tile([P, 2, SQ], F32, tag="pss", name="pss")
            nc.tensor.matmul(ps[:, 0, :], k8[j][0:DH, sl], q8[0:DH, j, :],
                             start=True, stop=True)
            nc.tensor.matmul(ps[:, 1, :], k8[j][DH:P, sl], q8[DH:P, j, :],
                             start=True, stop=True)
            nc.scalar.activation(e2[:, sti, :, :], ps[:], AF.Exp, scale=SCALE)
        e2s[(j, st2)] = e2

    def pv_acc(j, st2):
        if st2 == 0:
            pvs[j] = (pvp.tile([DH + 1, SQ], F32, tag="pv", name="pv"),
                      pvp.tile([DH + 1, SQ], F32, tag="pv", name="pv"))
        pv0, pv1 = pvs[j]
        e2 = e2s.pop((j, st2))
        nc.tensor.matmul(pv0[:], vr8[:, 2 * st2:2 * st2 + 2, 2 * j, :],
                         e2[:, :, 0, :], start=(st2 == 0),
                         stop=(st2 == ST // 2 - 1), perf_mode=DR)
        nc.tensor.matmul(pv1[:], vr8[:, 2 * st2:2 * st2 + 2, 2 * j + 1, :],
                         e2[:, :, 1, :], start=(st2 == 0),
                         stop=(st2 == ST // 2 - 1), perf_mode=DR)

    def pv_all(j):
        for st2 in range(4):
            pv_acc(j, st2)

    def norm(j):
        pv0, pv1 = pvs.pop(j)
        for half, pv in ((0, pv0), (1, pv1)):
            rows = slice(half * DH, half * DH + DH)
            zh = sc2.tile([1, SQ], F32, tag="zh", name="zh")
            nc.vector.tensor_copy(zh[:], pv[DH:DH + 1, :])
            iz = sc2.tile([1, SQ], F32, tag="zh", name="iz")
            nc.vector.reciprocal_approx_fast(iz[:], zh[:])
            bz = sc2.tile([DH, SQ], F32, tag="sb", name="sb")
            nc.gpsimd.partition_broadcast(bz[:], iz[:])
            nc.vector.tensor_mul(attn8[rows, j, :], pv[0:DH, :], bz[:])

    # ---- block schedule: K(j) first so exp starts ~9us in; Q pairs and V
    # groups fill the PE between the exp-gated scores/PV chains. V(c=0)
    # completes by block 2 (pv pairs 0-3 read heads 0-7), V(c=1) by block 5.
    # attn*V for pair p runs 1-2 blocks after its exps; e2 pool depth 8
    # covers the two in-flight pairs. ----
    wv0 = wload("wv", wv_ap[:, 0], [P, DT, SQ], FP8, nsplit=2)
    kproj(0, wk0)
    qpair(0, wqa, wqb)
    scx(0, 0)
    vh = v_pair_halves(0, 0, wv0)
    vh[0]()
    scx(0, 1)
    vh[1]()
    scx(0, 2)
    scx(0, 3)
    wv1 = None
    preload = {}
    for j in range(1, DT):
        w = wload("wka" if j % 2 == 0 else "wkb", wk_ap[:, j], [P, DT, P],
                  FP8, nsplit=1)
        if j == 3:
            wv1 = wload("wv", wv_ap[:, 1], [P, DT, SQ], FP8, nsplit=2)
        # filler work items (~0.9-1.8us each) to slot between the exp-gated
        # scores; c0 V pairs done by block 2, c1 by block 5
        items = []
        vg = {1: [(0, 2), (0, 4)], 2: [(0, 6)], 3: [(1, 0)], 4: [(1, 2)],
              5: [(1, 4), (1, 6)]}.get(j, [])
        for c, st in vg:
            items += v_pair_halves(c, st, wv0 if c == 0 else wv1)
        if j < 4:
            items += qpair_halves(
                2 * j,
                wload("wqa", wq_ap[:, 2 * j], [P, DT, P], BF16, nsplit=1),
                wload("wqb", wq_ap[:, 2 * j + 1], [P, DT, P], BF16,
                      nsplit=1))
        if j >= 2:
            items.append(lambda jj=j - 2: (pv_all(jj), norm(jj)))
        if j == 6:
            # prefetch the first Wo / W1 stationaries during late attention
            items.append(lambda: preload.update(
                woa=wload("woa", wo_ap[:, 0], [P, DT, P], FP8, nsplit=1),
                wob=wload("wob", wo_ap[:, 1], [P, DT, P], FP8, nsplit=1)))
        if j == 7:
            items.append(lambda: preload.update(
                w1a=wload("w1a", w1_ap[:, 0], [P, DT, P], BF16),
                w1b=wload("w1b", w1_ap[:, 1], [P, DT, P], BF16)))
        kproj(j, w)
        for s in range(4):
            scx(j, s)
            take = max(1, (len(items) + 3 - s) // (4 - s))
            for it in items[:take]:
                it()
            items = items[take:]
        for it in items:
            it()
    for j in range(DT - 2, DT):
        pv_all(j)
        norm(j)

    # ---- output projection (fp8 DoubleRow) + relu + residual + LN1 stats ----
    h1, sq1 = [], []
    h18 = acts.tile([P, DT, SQ], BF16, tag="h18", name="h18")
    ps_sum1 = pvp.tile([1, SQ], F32, tag="pv", name="ps_sum1")
    ps_sq1 = pvp.tile([1, SQ], F32, tag="pv", name="ps_sq1")

    def colsum(ps_sum, ps_sq, src, sq, j):
        nc.tensor.matmul(ps_sum[:], ones_r[:], src[:],
                         start=(j == 0), stop=(j == DT - 1))
        nc.tensor.matmul(ps_sq[:], ones_b[:], sq[:],
                         start=(j == 0), stop=(j == DT - 1))

    for j0 in range(0, DT, 2):
        wa = preload["woa"] if j0 == 0 else wload(
            "woa", wo_ap[:, j0], [P, DT, P], FP8, nsplit=1)
        wb = preload["wob"] if j0 == 0 else wload(
            "wob", wo_ap[:, j0 + 1], [P, DT, P], FP8, nsplit=1)
        ps = pss.tile([P, 2, SQ], F32, tag="pss", name="ps_wo")
        for k in range(0, DT, 2):
            nc.tensor.matmul(ps[:, 0, :], wa[:, k:k + 2, :],
                             attn8[:, k:k + 2, :],
                             start=(k == 0), stop=(k == DT - 2), perf_mode=DR)
            nc.tensor.matmul(ps[:, 1, :], wb[:, k:k + 2, :],
                             attn8[:, k:k + 2, :],
                             start=(k == 0), stop=(k == DT - 2), perf_mode=DR)
        # lag-one interleave of the LN1 column sums
        for j in (j0 - 2, j0 - 1):
            if j >= 0:
                colsum(ps_sum1, ps_sq1, h1[j], sq1[j], j)
        for h in range(2):
            j = j0 + h
            rel = sc2.tile([P, SQ], F32R, tag="u", name="rel")
            nc.scalar.activation(rel[:], ps[:, h, :], AF.Relu,
                                 bias=consts[:, _C_BO + j:_C_BO + j + 1],
                                 scale=1.0 / ALPHA)
            t = acts.tile([P, SQ], F32R, tag=f"h1{j}", name=f"h1_{j}")
            nc.vector.tensor_add(t[:], rel[:], q_res[j][:])
            h1.append(t)
            nc.scalar.copy(h18[:, j, :], t[:])
            sq = acts.tile([P, SQ], BF16, tag=f"sq1{j}", name=f"sq1_{j}")
            nc.vector.tensor_mul(sq[:], t[:], t[:])
            sq1.append(sq)
    for j in (DT - 2, DT - 1):
        colsum(ps_sum1, ps_sq1, h1[j], sq1[j], j)

    # LN1 chain (gamma/beta folded into W1 / rank-1 W2 fold)
    s_sb = sc1.tile([1, SQ], F32, tag="s0", name="s_sb")
    nc.vector.tensor_copy(s_sb[:], ps_sum1[:])
    m2 = sc1.tile([1, SQ], F32, tag="s1", name="m2")
    nc.vector.tensor_mul(m2[:], s_sb[:], s_sb[:])
    a_t = sc1.tile([1, SQ], F32, tag="s2", name="a_t")
    nc.vector.scalar_tensor_tensor(a_t[:], m2[:], 1.0 / D, ps_sq1[:],
                                   op0=mybir.AluOpType.mult,
                                   op1=mybir.AluOpType.subtract)
    eps_t = sc1.tile([1, 1], F32, tag="eps", name="eps")
    nc.vector.memset(eps_t[:], EPS)
    sd1 = sc1.tile([1, SQ], F32, tag="s1", name="sd1")
    nc.scalar.activation(sd1[:], a_t[:], AF.Sqrt, bias=eps_t[:], scale=-1.0 / D)
    rstd1 = sc1.tile([1, SQ], F32, tag="s2", name="rstd1")
    nc.vector.reciprocal_approx_fast(rstd1[:], sd1[:])
    bneg1 = sc1.tile([1, SQ], F32, tag="s3", name="bneg1")
    nc.vector.scalar_tensor_tensor(bneg1[:], s_sb[:], -1.0 / D, rstd1[:],
                                   op0=mybir.AluOpType.mult,
                                   op1=mybir.AluOpType.mult)
    negmu_r = sc1.tile([1, SQ], BF16, tag="s4", name="negmu_r")
    nc.vector.tensor_scalar_mul(negmu_r[:], s_sb[:], -1.0 / D)
    sd_r = sc1.tile([1, SQ], BF16, tag="s5", name="sd_r")
    nc.vector.tensor_copy(sd_r[:], sd1[:])
    abc_sb = sc2.tile([P, SQ], F32, tag="sb", name="abc_sb")
    nc.gpsimd.partition_broadcast(abc_sb[:], rstd1[:])
    bbc_sb = sc2.tile([P, SQ], F32, tag="zh", name="bbc_sb")
    nc.gpsimd.partition_broadcast(bbc_sb[:], bneg1[:])

    # ---- FFN1 (bf16) ----
    hid_a = acts.tile([P, DT, SQ], BF16, tag="xtb_hi", name="hid_a")
    hid_b = acts.tile([P, DT, SQ], BF16, tag="hid_b", name="hid_b")
    for f0 in range(0, FT, 2):
        wa = preload["w1a"] if f0 == 0 else wload(
            "w1a", w1_ap[:, f0], [P, DT, P], BF16)
        wb = preload["w1b"] if f0 == 0 else wload(
            "w1b", w1_ap[:, f0 + 1], [P, DT, P], BF16)
        ps = pss.tile([P, 2, SQ], F32, tag="pss", name="ps_f1")
        for k in range(DT):
            nc.tensor.matmul(ps[:, 0, :], wa[:, k, :], h18[:, k, :],
                             start=(k == 0), stop=(k == DT - 1))
            nc.tensor.matmul(ps[:, 1, :], wb[:, k, :], h18[:, k, :],
                             start=(k == 0), stop=(k == DT - 1))
        hid = hid_a if f0 < DT else hid_b
        nc.scalar.copy(hid[:, f0 % DT:f0 % DT + 2, :], ps[:])

    # real ln1 for the residual (overlaps FFN1)
    ln1 = []
    for j in range(DT):
        u = sc2.tile([P, SQ], F32, tag="u", name="u")
        nc.vector.tensor_mul(u[:], h1[j][:], abc_sb[:])
        nc.vector.tensor_add(u[:], u[:], bbc_sb[:])
        d = acts.tile([P, SQ], F32R, tag=f"ln1{j}", name=f"ln1_{j}")
        nc.scalar.activation(d[:], u[:], AF.Identity,
                             bias=consts[:, _C_BE1 + j:_C_BE1 + j + 1],
                             scale=consts[:, _C_G1 + j:_C_G1 + j + 1])
        ln1.append(d)

    # ---- FFN2 (bf16) + rank-1 LN1 fold + relu + residual + LN2 stats ----
    f2, sq2 = [], []
    ps_sum2 = pvp.tile([1, SQ], F32, tag="pv", name="ps_sum2")
    ps_sq2 = pvp.tile([1, SQ], F32, tag="pv", name="ps_sq2")
    for j in range(DT):
        w = wload("w2", w2_ap[:, j], [P, FT, P], BF16, nsplit=4)
        ps = pss.tile([P, 2, SQ], F32, tag="pss", name="ps_f2")
        for f in range(FT):
            hid = hid_a if f < DT else hid_b
            nc.tensor.matmul(ps[:, 0, :], w[:, f, :], hid[:, f % DT, :],
                             start=(f == 0), stop=False)
        nc.tensor.matmul(ps[:, 0, :], fold[0:1, j * P:(j + 1) * P],
                         negmu_r[:], start=False, stop=False)
        nc.tensor.matmul(ps[:, 0, :], fold[0:1, D + j * P:D + (j + 1) * P],
                         sd_r[:], start=False, stop=True)
        if j > 0:
            colsum(ps_sum2, ps_sq2, f2[j - 1], sq2[j - 1], j - 1)
        rel = sc2.tile([P, SQ], F32R, tag="u", name="rel2")
        nc.vector.scalar_tensor_tensor(rel[:], ps[:, 0, :], 0.0, abc_sb[:],
                                       op0=mybir.AluOpType.max,
                                       op1=mybir.AluOpType.mult)
        t = acts.tile([P, SQ], F32R, tag=f"h1{j}", name=f"f2_{j}")
        nc.vector.tensor_add(t[:], rel[:], ln1[j][:])
        f2.append(t)
        sq = acts.tile([P, SQ], BF16, tag=f"sq1{j}", name=f"sq2_{j}")
        nc.scalar.activation(sq[:], t[:], AF.Square)
        sq2.append(sq)
    colsum(ps_sum2, ps_sq2, f2[DT - 1], sq2[DT - 1], DT - 1)

    # ---- LN2 chain + normalize (writes bf16, DMA per tile) ----
    s_sb2 = sc1.tile([1, SQ], F32, tag="s0", name="s_sb2")
    nc.vector.tensor_copy(s_sb2[:], ps_sum2[:])
    m22 = sc1.tile([1, SQ], F32, tag="s1", name="m22")
    nc.vector.tensor_mul(m22[:], s_sb2[:], s_sb2[:])
    a_t2 = sc1.tile([1, SQ], F32, tag="s2", name="a_t2")
    nc.vector.scalar_tensor_tensor(a_t2[:], m22[:], 1.0 / D, ps_sq2[:],
                                   op0=mybir.AluOpType.mult,
                                   op1=mybir.AluOpType.subtract)
    sd2 = sc1.tile([1, SQ], F32, tag="s1", name="sd2")
    nc.scalar.activation(sd2[:], a_t2[:], AF.Sqrt, bias=eps_t[:],
                         scale=-1.0 / D)
    rstd2 = sc1.tile([1, SQ], F32, tag="s2", name="rstd2")
    nc.vector.reciprocal_approx_fast(rstd2[:], sd2[:])
    bneg2 = sc1.tile([1, SQ], F32R, tag="s3", name="bneg2")
    nc.vector.scalar_tensor_tensor(bneg2[:], s_sb2[:], -1.0 / D, rstd2[:],
                                   op0=mybir.AluOpType.mult,
                                   op1=mybir.AluOpType.mult)
    a_r = sc1.tile([1, SQ], F32R, tag="s0", name="a_r2")
    nc.vector.tensor_copy(a_r[:], rstd2[:])
    ab = pss.tile([P, 2, SQ], F32, tag="pss", name="ab")
    nc.tensor.matmul(ab[:, 0, :], ones_row[:], a_r[:], start=True, stop=True)
    nc.tensor.matmul(ab[:, 1, :], ones_row[:], bneg2[:], start=True, stop=True)
    for j in range(DT):
        u = sc2.tile([P, SQ], F32, tag="u", name="u")
        nc.vector.tensor_mul(u[:], f2[j][:], ab[:, 0, :])
        nc.vector.tensor_add(u[:], u[:], ab[:, 1, :])
        d = acts.tile([P, SQ], BF16, tag=f"qres{j}", name=f"y_{j}")
        nc.scalar.activation(d[:], u[:], AF.Identity,
                             bias=consts[:, _C_BE2 + j:_C_BE2 + j + 1],
                             scale=consts[:, _C_G2 + j:_C_G2 + j + 1])
        nc.sync.dma_start(yT_ap[j * P:(j + 1) * P, :], d[:])


def build():
    nc = bacc.Bacc("TRN2", target_bir_lowering=False, debug=False,
                   num_devices=NCORES)
    xT_ap = nc.dram_tensor("xT", [P, DT, S], BF16, kind="ExternalInput").ap()
    x8_ap = nc.dram_tensor("xT8", [P, DT, S], FP8, kind="ExternalInput").ap()
    wq_ap = nc.dram_tensor("Wq", [P, DT, DT, P], BF16, kind="ExternalInput").ap()
    wk_ap = nc.dram_tensor("Wk", [P, DT, DT, P], FP8, kind="ExternalInput").ap()
    wv_ap = nc.dram_tensor("Wv", [P, 2, DT, SQ], FP8, kind="ExternalInput").ap()
    wo_ap = nc.dram_tensor("Wo", [P, DT, DT, P], FP8, kind="ExternalInput").ap()
    w1_ap = nc.dram_tensor("W1", [P, FT, DT, P], BF16, kind="ExternalInput").ap()
    w2_ap = nc.dram_tensor("W2", [P, DT, FT, P], BF16, kind="ExternalInput").ap()
    consts_ap = nc.dram_tensor("consts", [P, 64], F32, kind="ExternalInput").ap()
    ones_ap = nc.dram_tensor("ones", [P, 1], F32R, kind="ExternalInput").ap()
    onesrow_ap = nc.dram_tensor("ones_row", [1, P], F32R, kind="ExternalInput").ap()
    fold_ap = nc.dram_tensor("fold", [1, 2 * D], FP8, kind="ExternalInput").ap()
    yT_ap = nc.dram_tensor("yT", [D, SQ], BF16, kind="ExternalOutput").ap()
    aps = (xT_ap, x8_ap, wq_ap, wk_ap, wv_ap, wo_ap, w1_ap, w2_ap,
           consts_ap, ones_ap, onesrow_ap, fold_ap, yT_ap)
    from contextlib import ExitStack
    with tile.TileContext(nc) as tc, ExitStack() as ctx:
        _emit(ctx, tc, aps)
    nc.compile()
    return nc


_cached_nc = None


def _get_nc():
    global _cached_nc
    if _cached_nc is None:
        _cached_nc = build()
    return _cached_nc


def _to_bf16(a):
    return np.ascontiguousarray(np.asarray(a, np.float32)).astype(
        ml_dtypes.bfloat16)


def _to_fp8(a, scale):
    return np.clip(np.asarray(a, np.float32) * scale, -240.0, 240.0).astype(
        ml_dtypes.float8_e4m3)


def _prep_in_maps(x, Wq, Wk, Wv, Wo, bo, ln1_g, ln1_b, W1, b1, W2, b2,
                  ln2_g, ln2_b):
    f = np.float32
    consts = np.zeros((P, 64), f)
    consts[:, _C_BO:_C_BO + 8] = np.asarray(bo, f).reshape(8, P).T
    consts[:, _C_B1:_C_B1 + 16] = np.asarray(b1, f).reshape(16, P).T
    consts[:, _C_B2:_C_B2 + 8] = np.asarray(b2, f).reshape(8, P).T
    consts[:, _C_G1:_C_G1 + 8] = np.asarray(ln1_g, f).reshape(8, P).T
    consts[:, _C_BE1:_C_BE1 + 8] = np.asarray(ln1_b, f).reshape(8, P).T
    consts[:, _C_G2:_C_G2 + 8] = np.asarray(ln2_g, f).reshape(8, P).T
    consts[:, _C_BE2:_C_BE2 + 8] = np.asarray(ln2_b, f).reshape(8, P).T
    ones = np.ones((P, 1), f)
    ones_row = np.ones((1, P), f)
    W1f = np.asarray(W1, np.float64)
    W2f = np.asarray(W2, np.float64)
    g1v = np.asarray(ln1_g, np.float64)
    b1v = np.asarray(ln1_b, np.float64)
    c1 = np.asarray(b1, np.float64) + (b1v[:, None] * W1f).sum(axis=0)
    W1g = (g1v[:, None] * W1f).astype(f)
    w2g1 = (g1v[:, None] * W1f).sum(axis=0) @ W2f
    c2 = np.asarray(b2, np.float64) + c1 @ W2f
    fold = np.concatenate([w2g1, c2]).astype(f)[None, :]

    def pack_st(W, dtype_fn):
        # [D_in, N] -> [P, N/P, D_in/P, P] stationary tiles
        din, n = W.shape
        return np.ascontiguousarray(
            dtype_fn(np.asarray(W, f).reshape(din // P, P, n // P, P)
                     .transpose(1, 2, 0, 3)))

    shared = {
        "Wq": pack_st(np.asarray(Wq, f), _to_bf16),
        "Wk": pack_st(np.asarray(Wk, f), lambda a: _to_fp8(a, ALPHA)),
        "Wo": pack_st(np.asarray(Wo, f), lambda a: _to_fp8(a, ALPHA)),
        "W1": pack_st(W1g, _to_bf16),
        "W2": pack_st(np.asarray(W2, f), _to_bf16),
        "Wv": np.ascontiguousarray(
            _to_fp8(np.asarray(Wv, f).reshape(DT, P, 2, SQ)
                    .transpose(1, 2, 0, 3), ALPHA)),
        "consts": consts, "ones": ones, "ones_row": ones_row,
        "fold": _to_fp8(fold, 1.0),
    }
    xt = np.asarray(x, f).transpose(0, 2, 1)  # [B, D, S]
    in_maps = []
    for core in range(NCORES):
        b, off = core // 2, (core % 2) * SQ
        if off == 0:
            xrot = xt[b]
        else:
            xrot = np.concatenate([xt[b][:, off:], xt[b][:, :off]], axis=1)
        xr = xrot.reshape(DT, P, S).transpose(1, 0, 2)
        in_maps.append(dict(shared, xT=np.ascontiguousarray(_to_bf16(xr)),
                            xT8=np.ascontiguousarray(_to_fp8(xr, 1.0))))
    return in_maps


def run(inputs, trace=False, tmpdir=None):
    """Run the kernel on 8 cores. Returns (y, BassKernelResults)."""
    nc = _get_nc()
    in_maps = _prep_in_maps(
        inputs["x"], inputs["Wq"], inputs["Wk"], inputs["Wv"], inputs["Wo"],
        inputs["bo"], inputs["ln1_g"], inputs["ln1_b"], inputs["W1"],
        inputs["b1"], inputs["W2"], inputs["b2"], inputs["ln2_g"],
        inputs["ln2_b"])
    try:
        res = bass_utils.run_bass_kernel_spmd(nc, in_maps, list(range(NCORES)),
                                              trace=trace, tmpdir=tmpdir)
    except Exception:
        import time as _time
        _time.sleep(2.0)
        res = bass_utils.run_bass_kernel_spmd(nc, in_maps, list(range(NCORES)),
                                              trace=trace, tmpdir=tmpdir)
    y = np.empty((B, S, D), np.float32)
    for core in range(NCORES):
        b, off = core // 2, (core % 2) * SQ
        y[b, off:off + SQ, :] = res.results[core]["yT"].astype(np.float32).T
    return y, res


def kernel(x, mask, Wq, Wk, Wv, Wo, bo, ln1_g, ln1_b, W1, b1, W2, b2,
           ln2_g, ln2_b):
    # mask is all-ones per the problem spec -> identity in the reference.
    y, _ = run(dict(x=x, Wq=Wq, Wk=Wk, Wv=Wv, Wo=Wo, bo=bo, ln1_g=ln1_g,
                    ln1_b=ln1_b, W1=W1, b1=b1, W2=W2, b2=b2, ln2_g=ln2_g,
                    ln2_b=ln2_b))
    return y


# revision 28
# speedup vs baseline: 1.0241x; 1.0241x over previous
"""Trainium2 Bass kernel for nn_EncoderLayer (B=4, S=1024, D=1024, H=16, FF=2048).

Sharding: 8 cores = 4 batches x 2 sequence-halves, each core redundantly
computes K/V for its whole batch and runs the layer for its own 512 query
rows. Odd cores see the sequence rotated by 512 (softmax over keys is
permutation-invariant).

Mixed-precision PE pipeline:
  - K/V projections, QK^T scores, exp, attn*V and the output projection run
    in fp8e4 (TRN E4M3) with MatmulPerfMode.DoubleRow (two 128-row
    contraction tiles per instruction = 2x PE throughput). fp8 weights are
    pre-scaled by 32 on the host; the 1/32 is folded into PSUM->SBUF copies.
  - Q projection and both FFN matmuls stay bf16: their error lands on the
    residual stream where fp8 blows the 2e-2 budget (measured 5e-2 all-fp8
    vs 3.2e-3 with these three in bf16).

Schedule: the softmax exp stream (~57us on the scalar engine) is the
second-longest pole after the PE, so scores for head-pair j are emitted
immediately after the K projection of tile j, with attn*V lagging one pair
(PV(j-1) fills the PE while the vector engine re-quantizes k8[j]). V(c=0)
is interleaved into the j=0 block, V(c=1) into the j=4 block (attn*V for
pairs 0-3 only reads heads 0-7 = the c=0 half of V).

Layernorm plumbing (feature-major activations, stats via ones-column
matmuls, gamma/beta folded into W1 + rank-1 W2 corrections) as in v1; the
LN column-sum matmuls are interleaved (lag-one) into the Wo/FFN2 loops and
the LN2 normalize reads the A/B broadcast PSUM directly and writes bf16.
"""

import sys
import types

import numpy as np
import ml_dtypes


def _shim_axon_hooks():
    try:
        import antenv.axon_hooks  # noqa: F401
    except Exception:
        mod = types.ModuleType("antenv.axon_hooks")
        mod.get_axon_ntff_profile_hook = lambda: None
        mod.set_axon_ntff_profile_hook = lambda h: None
        sys.modules["antenv.axon_hooks"] = mod


_shim_axon_hooks()

from concourse import bacc, mybir, tile  # noqa: E402
from concourse import bass_utils  # noqa: E402

F32 = mybir.dt.float32
F32R = mybir.dt.float32r
BF16 = mybir.dt.bfloat16
FP8 = mybir.dt.float8e4
AF = mybir.ActivationFunctionType
DR = mybir.MatmulPerfMode.DoubleRow

B, S, D, H, DH, FF = 4, 1024, 1024, 16, 64, 2048
SQ = 512
P = 128
DT = D // P
FT = FF // P
ST = S // P
NCORES = 8
EPS = 1e-6
SCALE = 1.0 / 32.0
ALPHA = 32.0

_C_BO = 0
_C_B1 = 8
_C_B2 = 24
_C_G1 = 32
_C_BE1 = 40
_C_G2 = 48
_C_BE2 = 56


def _emit(ctx, tc, aps):
    nc = tc.nc
    (xT_ap, x8_ap, wq_ap, wk_ap, wv_ap, wo_ap, w1_ap, w2_ap, consts_ap,
     ones_ap, onesrow_ap, fold_ap, yT_ap) = aps

    acts = ctx.enter_context(tc.tile_pool(name="acts", bufs=1))
    wts = ctx.enter_context(tc.tile_pool(name="wts", bufs=2))
    e2p = ctx.enter_context(tc.tile_pool(name="e2p", bufs=7))
    sc2 = ctx.enter_context(tc.tile_pool(name="sc2", bufs=2))
    sc1 = ctx.enter_context(tc.tile_pool(name="sc1", bufs=1))
    pp = ctx.enter_context(tc.tile_pool(name="pp", bufs=1, space="PSUM"))
    pss = ctx.enter_context(tc.tile_pool(name="pss", bufs=2, space="PSUM"))
    pvp = ctx.enter_context(tc.tile_pool(name="pvp", bufs=2, space="PSUM"))

    def wload(tag, src_ap, shape, dtype, nsplit=2):
        w = wts.tile(shape, dtype, tag=tag, name=tag)
        step = shape[1] // nsplit
        for q in range(nsplit):
            nc.sync.dma_start(w[:, q * step:(q + 1) * step, :],
                              src_ap[:, q * step:(q + 1) * step, :])
        return w

    # ---- input DMA: xt8 + wk0 first (K0 gates the exp stream), then Q's ----
    xt8_a = acts.tile([P, 4, S], FP8, tag="xt8", name="xt8_a")
    for q in range(4):
        nc.sync.dma_start(xt8_a[:, q:q + 1, :], x8_ap[:, q:q + 1, :])
    xt8_b = acts.tile([P, 4, S], FP8, tag="xt8b", name="xt8_b")
    for q in range(4):
        nc.sync.dma_start(xt8_b[:, q:q + 1, :], x8_ap[:, 4 + q:5 + q, :])

    def x8(k):
        return (xt8_a, k) if k < 4 else (xt8_b, k - 4)
    wk0 = wload("wka", wk_ap[:, 0], [P, DT, P], FP8)
    wqa = wload("wqa", wq_ap[:, 0], [P, DT, P], BF16)
    wqb = wload("wqb", wq_ap[:, 1], [P, DT, P], BF16)
    xtb_a = acts.tile([P, 4, SQ], BF16, tag="xtb_a", name="xtb_a")
    nc.sync.dma_start(xtb_a[:, 0:2, :], xT_ap[:, 0:2, 0:SQ])
    nc.sync.dma_start(xtb_a[:, 2:4, :], xT_ap[:, 2:4, 0:SQ])
    xtb_b = acts.tile([P, 4, SQ], BF16, tag="xtb_b", name="xtb_b")
    nc.sync.dma_start(xtb_b[:, 0:2, :], xT_ap[:, 4:6, 0:SQ])
    nc.sync.dma_start(xtb_b[:, 2:4, :], xT_ap[:, 6:8, 0:SQ])
    consts = acts.tile([P, 64], F32, tag="consts", name="consts")
    nc.sync.dma_start(consts[:], consts_ap[:])
    ones_r = acts.tile([P, 1], F32R, tag="ones", name="ones")
    nc.sync.dma_start(ones_r[:], ones_ap[:])
    ones_row = acts.tile([1, P], F32R, tag="ones_row", name="ones_row")
    nc.sync.dma_start(ones_row[:], onesrow_ap[:])
    ones_b = acts.tile([P, 1], BF16, tag="ones_b", name="ones_b")
    nc.vector.memset(ones_b[:], 1.0)
    fold = acts.tile([1, 2 * D], FP8, tag="fold", name="fold")
    nc.sync.dma_start(fold[:], fold_ap[:])

    def xlo(k):
        return xtb_a[:, k, :] if k < 4 else xtb_b[:, k - 4, :]

    # ---- attention machinery ----
    q_res = []
    q8 = acts.tile([P, DT, SQ], FP8, tag="q8", name="q8")
    vr8 = acts.tile([P, ST, H, DH + 1], FP8, tag="vr8", name="vr8")
    nc.vector.memset(vr8[:, :, :, DH:DH + 1], 1.0)
    attn8 = acts.tile([P, DT, SQ], FP8, tag="attn8", name="attn8")
    k8 = [None] * DT
    e2s = {}
    pvs = {}

    def qpair_halves(j0, wa, wb):
        """Returns two emission closures (k 0:4 and k 4:8 + copies)."""
        box = {}

        def half(lo, hi):
            if lo == 0:
                box["ps"] = pp.tile([P, 2, SQ], F32, tag="ps2", name="ps2")
            ps = box["ps"]
            for k in range(lo, hi):
                nc.tensor.matmul(ps[:, 0, :], wa[:, k, :], xlo(k),
                                 start=(k == 0), stop=(k == DT - 1))
                nc.tensor.matmul(ps[:, 1, :], wb[:, k, :], xlo(k),
                                 start=(k == 0), stop=(k == DT - 1))
            if hi == DT:
                for h in range(2):
                    j = j0 + h
                    qr = acts.tile([P, SQ], F32R, tag=f"qres{j}",
                                   name=f"qres{j}")
                    nc.scalar.copy(qr[:], ps[:, h, :])
                    q_res.append(qr)
                    nc.vector.tensor_copy(q8[:, j, :], ps[:, h, :])

        return [lambda: half(0, 4), lambda: half(4, DT)]

    def qpair(j0, wa, wb):
        for f in qpair_halves(j0, wa, wb):
            f()

    def kproj(j, w):
        ps = pp.tile([P, 2, SQ], F32, tag="ps2", name="ps2")
        for k in range(0, DT, 2):
            xt, kk = x8(k)
            nc.tensor.matmul(ps[:, 0, :], w[:, k:k + 2, :],
                             xt[:, kk:kk + 2, 0:SQ],
                             start=(k == 0), stop=(k == DT - 2), perf_mode=DR)
            nc.tensor.matmul(ps[:, 1, :], w[:, k:k + 2, :],
                             xt[:, kk:kk + 2, SQ:S],
                             start=(k == 0), stop=(k == DT - 2), perf_mode=DR)
        kj = acts.tile([P, S], FP8, tag=f"k8{j}", name=f"k8{j}")
        nc.vector.tensor_scalar_mul(
            kj[:].rearrange("p (c q) -> p c q", c=2), ps[:], 1.0 / ALPHA)
        k8[j] = kj

    def v_pair_halves(c, st0, wv):
        """Two emission closures, one per key tile; one psum tile + copy."""
        box = {}

        def half(si):
            if si == 0:
                box["ps"] = pp.tile([P, 2, SQ], F32, tag="ps2", name="ps2")
            ps = box["ps"]
            for k in range(0, DT, 2):
                xt, kk = x8(k)
                nc.tensor.matmul(
                    ps[:, si, :],
                    xt[:, kk:kk + 2, (st0 + si) * P:(st0 + si + 1) * P],
                    wv[:, k:k + 2, :],
                    start=(k == 0), stop=(k == DT - 2), perf_mode=DR)
            if si == 1:
                nc.vector.tensor_scalar_mul(
                    vr8[:, st0:st0 + 2, c * 8:(c + 1) * 8, 0:DH],
                    ps[:].rearrange("p s (h d) -> p s h d", d=DH),
                    1.0 / ALPHA)

        return [lambda: half(0), lambda: half(1)]

    def scx(j, st2):
        """scores + exp for head pair (2j, 2j+1), key tiles 2*st2, 2*st2+1."""
        e2 = e2p.tile([P, 2, 2, SQ], FP8, tag="e2", name="e2")
        for sti in range(2):
            st = 2 * st2 + sti
            sl = slice(st * P, (st + 1) * P)
            ps = pss.tile([P, 2, SQ], F32, tag="pss", name="pss")
            nc.tensor.matmul(ps[:, 0, :], k8[j][0:DH, sl], q8[0:DH, j, :],
                             start=True, stop=True)
            nc.tensor.matmul(ps[:, 1, :], k8[j][DH:P, sl], q8[DH:P, j, :],
                             start=True, stop=True)
            nc.scalar.activation(e2[:, sti, :, :], ps[:], AF.Exp, scale=SCALE)
        e2s[(j, st2)] = e2

    def pv_acc(j, st2):
        if st2 == 0:
            pvs[j] = (pvp.tile([DH + 1, SQ], F32, tag="pv", name="pv"),
                      pvp.tile([DH + 1, SQ], F32, tag="pv", name="pv"))
        pv0, pv1 = pvs[j]
        e2 = e2s.pop((j, st2))
        nc.tensor.matmul(pv0[:], vr8[:, 2 * st2:2 * st2 + 2, 2 * j, :],
                         e2[:, :, 0, :], start=(st2 == 0),
                         stop=(st2 == ST // 2 - 1), perf_mode=DR)
        nc.tensor.matmul(pv1[:], vr8[:, 2 * st2:2 * st2 + 2, 2 * j + 1, :],
                         e2[:, :, 1, :], start=(st2 == 0),
                         stop=(st2 == ST // 2 - 1), perf_mode=DR)

    def pv_all(j):
        for st2 in range(4):
            pv_acc(j, st2)

    def norm(j):
        pv0, pv1 = pvs.pop(j)
        for half, pv in ((0, pv0), (1, pv1)):
            rows = slice(half * DH, half * DH + DH)
            zh = sc2.tile([1, SQ], F32, tag="zh", name="zh")
            nc.vector.tensor_copy(zh[:], pv[DH:DH + 1, :])
            iz = sc2.tile([1, SQ], F32, tag="zh", name="iz")
            nc.vector.reciprocal_approx_fast(iz[:], zh[:])
            bz = sc2.tile([DH, SQ], F32, tag="sb", name="sb")
            nc.gpsimd.partition_broadcast(bz[:], iz[:])
            nc.vector.tensor_mul(attn8[rows, j, :], pv[0:DH, :], bz[:])

    # ---- block schedule: K(j) first so exp starts ~9us in; Q pairs and V
    # groups fill the PE between the exp-gated scores/PV chains. V(c=0)
    # completes by block 2 (pv pairs 0-3 read heads 0-7), V(c=1) by block 5.
    # attn*V for pair p runs 1-2 blocks after its exps; e2 pool depth 8
    # covers the two in-flight pairs. ----
    wv0 = wload("wv", wv_ap[:, 0], [P, DT, SQ], FP8, nsplit=2)
    kproj(0, wk0)
    qpair(0, wqa, wqb)
    scx(0, 0)
    vh = v_pair_halves(0, 0, wv0)
    vh[0]()
    scx(0, 1)
    vh[1]()
    scx(0, 2)
    scx(0, 3)
    wv1 = None
    preload = {}
    for j in range(1, DT):
        w = wload("wka" if j % 2 == 0 else "wkb", wk_ap[:, j], [P, DT, P],
                  FP8, nsplit=1)
        if j == 3:
            wv1 = wload("wv", wv_ap[:, 1], [P, DT, SQ], FP8, nsplit=2)
        # filler work items (~0.9-1.8us each) to slot between the exp-gated
        # scores; c0 V pairs done by block 2, c1 by block 5
        items = []
        vg = {1: [(0, 2), (0, 4)], 2: [(0, 6)], 3: [(1, 0)], 4: [(1, 2)],
              5: [(1, 4), (1, 6)]}.get(j, [])
        for c, st in vg:
            items += v_pair_halves(c, st, wv0 if c == 0 else wv1)
        if j < 4:
            items += qpair_halves(
                2 * j,
                wload("wqa", wq_ap[:, 2 * j], [P, DT, P], BF16, nsplit=1),
                wload("wqb", wq_ap[:, 2 * j + 1], [P, DT, P], BF16,
                      nsplit=1))
        if j >= 2:
            items.append(lambda jj=j - 2: (pv_all(jj), norm(jj)))
        if j == 6:
            # prefetch the first Wo / W1 stationaries during late attention
            items.append(lambda: preload.update(
                woa=wload("woa", wo_ap[:, 0], [P, DT, P], FP8, nsplit=1),
                wob=wload("wob", wo_ap[:, 1], [P, DT, P], FP8, nsplit=1)))
        if j == 7:
            items.append(lambda: preload.update(
                w1a=wload("w1a", w1_ap[:, 0], [P, DT, P], BF16),
                w1b=wload("w1b", w1_ap[:, 1], [P, DT, P], BF16)))
        kproj(j, w)
        for s in range(4):
            scx(j, s)
            take = max(1, (len(items) + 3 - s) // (4 - s))
            for it in items[:take]:
                it()
            items = items[take:]
        for it in items:
            it()
    for j in range(DT - 2, DT):
        pv_all(j)
        norm(j)

    # ---- output projection (fp8 DoubleRow) + relu + residual + LN1 stats ----
    h1, sq1 = [], []
    h18 = acts.tile([P, DT, SQ], BF16, tag="h18", name="h18")
    ps_sum1 = pvp.tile([1, SQ], F32, tag="pv", name="ps_sum1")
    ps_sq1 = pvp.tile([1, SQ], F32, tag="pv", name="ps_sq1")

    def colsum(ps_sum, ps_sq, src, sq, j):
        nc.tensor.matmul(ps_sum[:], ones_r[:], src[:],
                         start=(j == 0), stop=(j == DT - 1))
        nc.tensor.matmul(ps_sq[:], ones_b[:], sq[:],
                         start=(j == 0), stop=(j == DT - 1))

    for j0 in range(0, DT, 2):
        wa = preload["woa"] if j0 == 0 else wload(
            "woa", wo_ap[:, j0], [P, DT, P], FP8, nsplit=1)
        wb = preload["wob"] if j0 == 0 else wload(
            "wob", wo_ap[:, j0 + 1], [P, DT, P], FP8, nsplit=1)
        ps = pss.tile([P, 2, SQ], F32, tag="pss", name="ps_wo")
        for k in range(0, DT, 2):
            nc.tensor.matmul(ps[:, 0, :], wa[:, k:k + 2, :],
                             attn8[:, k:k + 2, :],
                             start=(k == 0), stop=(k == DT - 2), perf_mode=DR)
            nc.tensor.matmul(ps[:, 1, :], wb[:, k:k + 2, :],
                             attn8[:, k:k + 2, :],
                             start=(k == 0), stop=(k == DT - 2), perf_mode=DR)
        # lag-one interleave of the LN1 column sums
        for j in (j0 - 2, j0 - 1):
            if j >= 0:
                colsum(ps_sum1, ps_sq1, h1[j], sq1[j], j)
        for h in range(2):
            j = j0 + h
            rel = sc2.tile([P, SQ], F32R, tag="u", name="rel")
            nc.scalar.activation(rel[:], ps[:, h, :], AF.Relu,
                                 bias=consts[:, _C_BO + j:_C_BO + j + 1],
                                 scale=1.0 / ALPHA)
            t = acts.tile([P, SQ], F32R, tag=f"h1{j}", name=f"h1_{j}")
            nc.vector.tensor_add(t[:], rel[:], q_res[j][:])
            h1.append(t)
            nc.scalar.copy(h18[:, j, :], t[:])
            sq = acts.tile([P, SQ], BF16, tag=f"sq1{j}", name=f"sq1_{j}")
            nc.vector.tensor_mul(sq[:], t[:], t[:])
            sq1.append(sq)
    for j in (DT - 2, DT - 1):
        colsum(ps_sum1, ps_sq1, h1[j], sq1[j], j)

    # LN1 chain (gamma/beta folded into W1 / rank-1 W2 fold)
    s_sb = sc1.tile([1, SQ], F32, tag="s0", name="s_sb")
    nc.vector.tensor_copy(s_sb[:], ps_sum1[:])
    m2 = sc1.tile([1, SQ], F32, tag="s1", name="m2")
    nc.vector.tensor_mul(m2[:], s_sb[:], s_sb[:])
    a_t = sc1.tile([1, SQ], F32, tag="s2", name="a_t")
    nc.vector.scalar_tensor_tensor(a_t[:], m2[:], 1.0 / D, ps_sq1[:],
                                   op0=mybir.AluOpType.mult,
                                   op1=mybir.AluOpType.subtract)
    eps_t = sc1.tile([1, 1], F32, tag="eps", name="eps")
    nc.vector.memset(eps_t[:], EPS)
    sd1 = sc1.tile([1, SQ], F32, tag="s1", name="sd1")
    nc.scalar.activation(sd1[:], a_t[:], AF.Sqrt, bias=eps_t[:], scale=-1.0 / D)
    rstd1 = sc1.tile([1, SQ], F32, tag="s2", name="rstd1")
    nc.vector.reciprocal_approx_fast(rstd1[:], sd1[:])
    bneg1 = sc1.tile([1, SQ], F32, tag="s3", name="bneg1")
    nc.vector.scalar_tensor_tensor(bneg1[:], s_sb[:], -1.0 / D, rstd1[:],
                                   op0=mybir.AluOpType.mult,
                                   op1=mybir.AluOpType.mult)
    negmu_r = sc1.tile([1, SQ], BF16, tag="s4", name="negmu_r")
    nc.vector.tensor_scalar_mul(negmu_r[:], s_sb[:], -1.0 / D)
    sd_r = sc1.tile([1, SQ], BF16, tag="s5", name="sd_r")
    nc.vector.tensor_copy(sd_r[:], sd1[:])
    abc_sb = sc2.tile([P, SQ], F32, tag="sb", name="abc_sb")
    nc.gpsimd.partition_broadcast(abc_sb[:], rstd1[:])
    bbc_sb = sc2.tile([P, SQ], F32, tag="zh", name="bbc_sb")
    nc.gpsimd.partition_broadcast(bbc_sb[:], bneg1[:])

    # ---- FFN1 (bf16) ----
    hid_a = acts.tile([P, DT, SQ], BF16, tag="xtb_hi", name="hid_a")
    hid_b = acts.tile([P, DT, SQ], BF16, tag="hid_b", name="hid_b")
    for f0 in range(0, FT, 2):
        wa = preload["w1a"] if f0 == 0 else wload(
            "w1a", w1_ap[:, f0], [P, DT, P], BF16)
        wb = preload["w1b"] if f0 == 0 else wload(
            "w1b", w1_ap[:, f0 + 1], [P, DT, P], BF16)
        ps = pss.tile([P, 2, SQ], F32, tag="pss", name="ps_f1")
        for k in range(DT):
            nc.tensor.matmul(ps[:, 0, :], wa[:, k, :], h18[:, k, :],
                             start=(k == 0), stop=(k == DT - 1))
            nc.tensor.matmul(ps[:, 1, :], wb[:, k, :], h18[:, k, :],
                             start=(k == 0), stop=(k == DT - 1))
        hid = hid_a if f0 < DT else hid_b
        nc.scalar.copy(hid[:, f0 % DT:f0 % DT + 2, :], ps[:])

    # real ln1 for the residual (overlaps FFN1)
    ln1 = []
    for j in range(DT):
        u = sc2.tile([P, SQ], F32, tag="u", name="u")
        nc.vector.tensor_mul(u[:], h1[j][:], abc_sb[:])
        nc.vector.tensor_add(u[:], u[:], bbc_sb[:])
        d = acts.tile([P, SQ], F32R, tag=f"ln1{j}", name=f"ln1_{j}")
        nc.scalar.activation(d[:], u[:], AF.Identity,
                             bias=consts[:, _C_BE1 + j:_C_BE1 + j + 1],
                             scale=consts[:, _C_G1 + j:_C_G1 + j + 1])
        ln1.append(d)

    # ---- FFN2 (bf16) + rank-1 LN1 fold + relu + residual + LN2 stats ----
    f2, sq2 = [], []
    ps_sum2 = pvp.tile([1, SQ], F32, tag="pv", name="ps_sum2")
    ps_sq2 = pvp.tile([1, SQ], F32, tag="pv", name="ps_sq2")
    for j in range(DT):
        w = wload("w2", w2_ap[:, j], [P, FT, P], BF16, nsplit=4)
        ps = pss.tile([P, 2, SQ], F32, tag="pss", name="ps_f2")
        for f in range(FT):
            hid = hid_a if f < DT else hid_b
            nc.tensor.matmul(ps[:, 0, :], w[:, f, :], hid[:, f % DT, :],
                             start=(f == 0), stop=False)
        nc.tensor.matmul(ps[:, 0, :], fold[0:1, j * P:(j + 1) * P],
                         negmu_r[:], start=False, stop=False)
        nc.tensor.matmul(ps[:, 0, :], fold[0:1, D + j * P:D + (j + 1) * P],
                         sd_r[:], start=False, stop=True)
        if j > 0:
            colsum(ps_sum2, ps_sq2, f2[j - 1], sq2[j - 1], j - 1)
        rel = sc2.tile([P, SQ], F32R, tag="u", name="rel2")
        nc.vector.scalar_tensor_tensor(rel[:], ps[:, 0, :], 0.0, abc_sb[:],
                                       op0=mybir.AluOpType.max,
                                       op1=mybir.AluOpType.mult)
        t = acts.tile([P, SQ], F32R, tag=f"h1{j}", name=f"f2_{j}")
        nc.vector.tensor_add(t[:], rel[:], ln1[j][:])
        f2.append(t)
        sq = acts.tile([P, SQ], BF16, tag=f"sq1{j}", name=f"sq2_{j}")
        nc.scalar.activation(sq[:], t[:], AF.Square)
        sq2.append(sq)
    colsum(ps_sum2, ps_sq2, f2[DT - 1], sq2[DT - 1], DT - 1)

    # ---- LN2 chain + normalize (writes bf16, DMA per tile) ----
    s_sb2 = sc1.tile([1, SQ], F32, tag="s0", name="s_sb2")
    nc.vector.tensor_copy(s_sb2[:], ps_sum2[:])
    m22 = sc1.tile([1, SQ], F32, tag="s1", name="m22")
    nc.vector.tensor_mul(m22[:], s_sb2[:], s_sb2[:])
    a_t2 = sc1.tile([1, SQ], F32, tag="s2", name="a_t2")
    nc.vector.scalar_tensor_tensor(a_t2[:], m22[:], 1.0 / D, ps_sq2[:],
                                   op0=mybir.AluOpType.mult,
                                   op1=mybir.AluOpType.subtract)
    sd2 = sc1.tile([1, SQ], F32, tag="s1", name="sd2")
    nc.scalar.activation(sd2[:], a_t2[:], AF.Sqrt, bias=eps_t[:],
                         scale=-1.0 / D)
    rstd2 = sc1.tile([1, SQ], F32, tag="s2", name="rstd2")
    nc.vector.reciprocal_approx_fast(rstd2[:], sd2[:])
    bneg2 = sc1.tile([1, SQ], F32R, tag="s3", name="bneg2")
    nc.vector.scalar_tensor_tensor(bneg2[:], s_sb2[:], -1.0 / D, rstd2[:],
                                   op0=mybir.AluOpType.mult,
                                   op1=mybir.AluOpType.mult)
    a_r = sc1.tile([1, SQ], F32R, tag="s0", name="a_r2")
    nc.vector.tensor_copy(a_r[:], rstd2[:])
    ab = pss.tile([P, 2, SQ], F32, tag="pss", name="ab")
    nc.tensor.matmul(ab[:, 0, :], ones_row[:], a_r[:], start=True, stop=True)
    nc.tensor.matmul(ab[:, 1, :], ones_row[:], bneg2[:], start=True, stop=True)
    for j in range(DT):
        u = sc2.tile([P, SQ], F32, tag="u", name="u")
        nc.vector.tensor_mul(u[:], f2[j][:], ab[:, 0, :])
        nc.vector.tensor_add(u[:], u[:], ab[:, 1, :])
        d = acts.tile([P, SQ], BF16, tag=f"qres{j}", name=f"y_{j}")
        nc.scalar.activation(d[:], u[:], AF.Identity,
                             bias=consts[:, _C_BE2 + j:_C_BE2 + j + 1],
                             scale=consts[:, _C_G2 + j:_C_G2 + j + 1])
        nc.sync.dma_start(yT_ap[j * P:(j + 1) * P, :], d[:])


def build():
    nc = bacc.Bacc("TRN2", target_bir_lowering=False, debug=False,
                   num_devices=NCORES)
    xT_ap = nc.dram_tensor("xT", [P, DT, S], BF16, kind="ExternalInput").ap()
    x8_ap = nc.dram_tensor("xT8", [P, DT, S], FP8, kind="ExternalInput").ap()
    wq_ap = nc.dram_tensor("Wq", [P, DT, DT, P], BF16, kind="ExternalInput").ap()
    wk_ap = nc.dram_tensor("Wk", [P, DT, DT, P], FP8, kind="ExternalInput").ap()
    wv_ap = nc.dram_tensor("Wv", [P, 2, DT, SQ], FP8, kind="ExternalInput").ap()
    wo_ap = nc.dram_tensor("Wo", [P, DT, DT, P], FP8, kind="ExternalInput").ap()
    w1_ap = nc.dram_tensor("W1", [P, FT, DT, P], BF16, kind="ExternalInput").ap()
    w2_ap = nc.dram_tensor("W2", [P, DT, FT, P], BF16, kind="ExternalInput").ap()
    consts_ap = nc.dram_tensor("consts", [P, 64], F32, kind="ExternalInput").ap()
    ones_ap = nc.dram_tensor("ones", [P, 1], F32R, kind="ExternalInput").ap()
    onesrow_ap = nc.dram_tensor("ones_row", [1, P], F32R, kind="ExternalInput").ap()
    fold_ap = nc.dram_tensor("fold", [1, 2 * D], FP8, kind="ExternalInput").ap()
    yT_ap = nc.dram_tensor("yT", [D, SQ], BF16, kind="ExternalOutput").ap()
    aps = (xT_ap, x8_ap, wq_ap, wk_ap, wv_ap, wo_ap, w1_ap, w2_ap,
           consts_ap, ones_ap, onesrow_ap, fold_ap, yT_ap)
    from contextlib import ExitStack
    with tile.TileContext(nc) as tc, ExitStack() as ctx:
        _emit(ctx, tc, aps)
    nc.compile()
    return nc


_cached_nc = None


def _get_nc():
    global _cached_nc
    if _cached_nc is None:
        _cached_nc = build()
    return _cached_nc


def _to_bf16(a):
    return np.ascontiguousarray(np.asarray(a, np.float32)).astype(
        ml_dtypes.bfloat16)


def _to_fp8(a, scale):
    return np.clip(np.asarray(a, np.float32) * scale, -240.0, 240.0).astype(
        ml_dtypes.float8_e4m3)


def _prep_in_maps(x, Wq, Wk, Wv, Wo, bo, ln1_g, ln1_b, W1, b1, W2, b2,
                  ln2_g, ln2_b):
    f = np.float32
    consts = np.zeros((P, 64), f)
    consts[:, _C_BO:_C_BO + 8] = np.asarray(bo, f).reshape(8, P).T
    consts[:, _C_B1:_C_B1 + 16] = np.asarray(b1, f).reshape(16, P).T
    consts[:, _C_B2:_C_B2 + 8] = np.asarray(b2, f).reshape(8, P).T
    consts[:, _C_G1:_C_G1 + 8] = np.asarray(ln1_g, f).reshape(8, P).T
    consts[:, _C_BE1:_C_BE1 + 8] = np.asarray(ln1_b, f).reshape(8, P).T
    consts[:, _C_G2:_C_G2 + 8] = np.asarray(ln2_g, f).reshape(8, P).T
    consts[:, _C_BE2:_C_BE2 + 8] = np.asarray(ln2_b, f).reshape(8, P).T
    ones = np.ones((P, 1), f)
    ones_row = np.ones((1, P), f)
    W1f = np.asarray(W1, np.float64)
    W2f = np.asarray(W2, np.float64)
    g1v = np.asarray(ln1_g, np.float64)
    b1v = np.asarray(ln1_b, np.float64)
    c1 = np.asarray(b1, np.float64) + (b1v[:, None] * W1f).sum(axis=0)
    W1g = (g1v[:, None] * W1f).astype(f)
    w2g1 = (g1v[:, None] * W1f).sum(axis=0) @ W2f
    c2 = np.asarray(b2, np.float64) + c1 @ W2f
    fold = np.concatenate([w2g1, c2]).astype(f)[None, :]

    def pack_st(W, dtype_fn):
        # [D_in, N] -> [P, N/P, D_in/P, P] stationary tiles
        din, n = W.shape
        return np.ascontiguousarray(
            dtype_fn(np.asarray(W, f).reshape(din // P, P, n // P, P)
                     .transpose(1, 2, 0, 3)))

    shared = {
        "Wq": pack_st(np.asarray(Wq, f), _to_bf16),
        "Wk": pack_st(np.asarray(Wk, f), lambda a: _to_fp8(a, ALPHA)),
        "Wo": pack_st(np.asarray(Wo, f), lambda a: _to_fp8(a, ALPHA)),
        "W1": pack_st(W1g, _to_bf16),
        "W2": pack_st(np.asarray(W2, f), _to_bf16),
        "Wv": np.ascontiguousarray(
            _to_fp8(np.asarray(Wv, f).reshape(DT, P, 2, SQ)
                    .transpose(1, 2, 0, 3), ALPHA)),
        "consts": consts, "ones": ones, "ones_row": ones_row,
        "fold": _to_fp8(fold, 1.0),
    }
    xt = np.asarray(x, f).transpose(0, 2, 1)  # [B, D, S]
    in_maps = []
    for core in range(NCORES):
        b, off = core // 2, (core % 2) * SQ
        if off == 0:
            xrot = xt[b]
        else:
            xrot = np.concatenate([xt[b][:, off:], xt[b][:, :off]], axis=1)
        xr = xrot.reshape(DT, P, S).transpose(1, 0, 2)
        in_maps.append(dict(shared, xT=np.ascontiguousarray(_to_bf16(xr)),
                            xT8=np.ascontiguousarray(_to_fp8(xr, 1.0))))
    return in_maps


def run(inputs, trace=False, tmpdir=None):
    """Run the kernel on 8 cores. Returns (y, BassKernelResults)."""
    nc = _get_nc()
    in_maps = _prep_in_maps(
        inputs["x"], inputs["Wq"], inputs["Wk"], inputs["Wv"], inputs["Wo"],
        inputs["bo"], inputs["ln1_g"], inputs["ln1_b"], inputs["W1"],
        inputs["b1"], inputs["W2"], inputs["b2"], inputs["ln2_g"],
        inputs["ln2_b"])
    try:
        res = bass_utils.run_bass_kernel_spmd(nc, in_maps, list(range(NCORES)),
                                              trace=trace, tmpdir=tmpdir)
    except Exception:
        import time as _time
        _time.sleep(2.0)
        res = bass_utils.run_bass_kernel_spmd(nc, in_maps, list(range(NCORES)),
                                              trace=trace, tmpdir=tmpdir)
    y = np.empty((B, S, D), np.float32)
    for core in range(NCORES):
        b, off = core // 2, (core % 2) * SQ
        y[b, off:off + SQ, :] = res.results[core]["yT"].astype(np.float32).T
    return y, res


def kernel(x, mask, Wq, Wk, Wv, Wo, bo, ln1_g, ln1_b, W1, b1, W2, b2,
           ln2_g, ln2_b):
    # mask is all-ones per the problem spec -> identity in the reference.
    y, _ = run(dict(x=x, Wq=Wq, Wk=Wk, Wv=Wv, Wo=Wo, bo=bo, ln1_g=ln1_g,
                    ln1_b=ln1_b, W1=W1, b1=b1, W2=W2, b2=b2, ln2_g=ln2_g,
                    ln2_b=ln2_b))
    return y


# revision 29
# speedup vs baseline: 1.0276x; 1.0034x over previous
"""Trainium2 Bass kernel for nn_EncoderLayer (B=4, S=1024, D=1024, H=16, FF=2048).

Sharding: 8 cores = 4 batches x 2 sequence-halves, each core redundantly
computes K/V for its whole batch and runs the layer for its own 512 query
rows. Odd cores see the sequence rotated by 512 (softmax over keys is
permutation-invariant).

Mixed-precision PE pipeline:
  - K/V projections, QK^T scores, exp, attn*V and the output projection run
    in fp8e4 (TRN E4M3) with MatmulPerfMode.DoubleRow (two 128-row
    contraction tiles per instruction = 2x PE throughput). fp8 weights are
    pre-scaled by 32 on the host; the 1/32 is folded into PSUM->SBUF copies.
  - Q projection and both FFN matmuls stay bf16: their error lands on the
    residual stream where fp8 blows the 2e-2 budget (measured 5e-2 all-fp8
    vs 3.2e-3 with these three in bf16).

Schedule: the softmax exp stream (~57us on the scalar engine) is the
second-longest pole after the PE, so scores for head-pair j are emitted
immediately after the K projection of tile j, with attn*V lagging one pair
(PV(j-1) fills the PE while the vector engine re-quantizes k8[j]). V(c=0)
is interleaved into the j=0 block, V(c=1) into the j=4 block (attn*V for
pairs 0-3 only reads heads 0-7 = the c=0 half of V).

Layernorm plumbing (feature-major activations, stats via ones-column
matmuls, gamma/beta folded into W1 + rank-1 W2 corrections) as in v1; the
LN column-sum matmuls are interleaved (lag-one) into the Wo/FFN2 loops and
the LN2 normalize reads the A/B broadcast PSUM directly and writes bf16.
"""

import sys
import types

import numpy as np
import ml_dtypes


def _shim_axon_hooks():
    try:
        import antenv.axon_hooks  # noqa: F401
    except Exception:
        mod = types.ModuleType("antenv.axon_hooks")
        mod.get_axon_ntff_profile_hook = lambda: None
        mod.set_axon_ntff_profile_hook = lambda h: None
        sys.modules["antenv.axon_hooks"] = mod


_shim_axon_hooks()

from concourse import bacc, mybir, tile  # noqa: E402
from concourse import bass_utils  # noqa: E402

F32 = mybir.dt.float32
F32R = mybir.dt.float32r
BF16 = mybir.dt.bfloat16
FP8 = mybir.dt.float8e4
AF = mybir.ActivationFunctionType
DR = mybir.MatmulPerfMode.DoubleRow

B, S, D, H, DH, FF = 4, 1024, 1024, 16, 64, 2048
SQ = 512
P = 128
DT = D // P
FT = FF // P
ST = S // P
NCORES = 8
EPS = 1e-6
SCALE = 1.0 / 32.0
ALPHA = 32.0

_C_BO = 0
_C_B1 = 8
_C_B2 = 24
_C_G1 = 32
_C_BE1 = 40
_C_G2 = 48
_C_BE2 = 56


def _emit(ctx, tc, aps):
    nc = tc.nc
    (xT_ap, x8_ap, wq_ap, wk_ap, wv_ap, wo_ap, w1_ap, w2_ap, consts_ap,
     ones_ap, onesrow_ap, fold_ap, yT_ap) = aps

    acts = ctx.enter_context(tc.tile_pool(name="acts", bufs=1))
    wts = ctx.enter_context(tc.tile_pool(name="wts", bufs=2))
    e2p = ctx.enter_context(tc.tile_pool(name="e2p", bufs=7))
    sc2 = ctx.enter_context(tc.tile_pool(name="sc2", bufs=2))
    sc1 = ctx.enter_context(tc.tile_pool(name="sc1", bufs=1))
    pp = ctx.enter_context(tc.tile_pool(name="pp", bufs=1, space="PSUM"))
    pss = ctx.enter_context(tc.tile_pool(name="pss", bufs=2, space="PSUM"))
    pvp = ctx.enter_context(tc.tile_pool(name="pvp", bufs=2, space="PSUM"))

    def wload(tag, src_ap, shape, dtype, nsplit=2):
        w = wts.tile(shape, dtype, tag=tag, name=tag)
        step = shape[1] // nsplit
        for q in range(nsplit):
            nc.sync.dma_start(w[:, q * step:(q + 1) * step, :],
                              src_ap[:, q * step:(q + 1) * step, :])
        return w

    # ---- input DMA: xt8 + wk0 first (K0 gates the exp stream), then Q's ----
    xt8 = acts.tile([P, DT, S], FP8, tag="xt8", name="xt8")
    for q in range(DT):
        nc.sync.dma_start(xt8[:, q:q + 1, :], x8_ap[:, q:q + 1, :])

    def x8(k):
        return (xt8, k)
    wk0 = wload("wka", wk_ap[:, 0], [P, DT, P], FP8)
    wqa = wload("wqa", wq_ap[:, 0], [P, DT, P], BF16)
    wqb = wload("wqb", wq_ap[:, 1], [P, DT, P], BF16)
    xtb_a = acts.tile([P, 4, SQ], BF16, tag="xtb_a", name="xtb_a")
    nc.sync.dma_start(xtb_a[:, 0:2, :], xT_ap[:, 0:2, 0:SQ])
    nc.sync.dma_start(xtb_a[:, 2:4, :], xT_ap[:, 2:4, 0:SQ])
    xtb_b = acts.tile([P, 4, SQ], BF16, tag="xtb_b", name="xtb_b")
    nc.sync.dma_start(xtb_b[:, 0:2, :], xT_ap[:, 4:6, 0:SQ])
    nc.sync.dma_start(xtb_b[:, 2:4, :], xT_ap[:, 6:8, 0:SQ])
    consts = acts.tile([P, 64], F32, tag="consts", name="consts")
    nc.sync.dma_start(consts[:], consts_ap[:])
    ones_r = acts.tile([P, 1], F32R, tag="ones", name="ones")
    nc.sync.dma_start(ones_r[:], ones_ap[:])
    ones_row = acts.tile([1, P], F32R, tag="ones_row", name="ones_row")
    nc.sync.dma_start(ones_row[:], onesrow_ap[:])
    ones_b = acts.tile([P, 1], BF16, tag="ones_b", name="ones_b")
    nc.vector.memset(ones_b[:], 1.0)
    fold = acts.tile([1, 2 * D], FP8, tag="fold", name="fold")
    nc.sync.dma_start(fold[:], fold_ap[:])

    def xlo(k):
        return xtb_a[:, k, :] if k < 4 else xtb_b[:, k - 4, :]

    # ---- attention machinery ----
    q_res = []
    q8 = acts.tile([P, DT, SQ], FP8, tag="q8", name="q8")
    vr8 = acts.tile([P, ST, H, DH + 1], FP8, tag="vr8", name="vr8")
    nc.vector.memset(vr8[:, :, :, DH:DH + 1], 1.0)
    attn8 = acts.tile([P, DT, SQ], FP8, tag="attn8", name="attn8")
    k8 = [None] * DT
    e2s = {}
    pvs = {}

    def qpair_halves(j0, wa, wb):
        """Returns two emission closures (k 0:4 and k 4:8 + copies)."""
        box = {}

        def half(lo, hi):
            if lo == 0:
                box["ps"] = pp.tile([P, 2, SQ], F32, tag="ps2", name="ps2")
            ps = box["ps"]
            for k in range(lo, hi):
                nc.tensor.matmul(ps[:, 0, :], wa[:, k, :], xlo(k),
                                 start=(k == 0), stop=(k == DT - 1))
                nc.tensor.matmul(ps[:, 1, :], wb[:, k, :], xlo(k),
                                 start=(k == 0), stop=(k == DT - 1))
            if hi == DT:
                for h in range(2):
                    j = j0 + h
                    qr = acts.tile([P, SQ], F32R, tag=f"qres{j}",
                                   name=f"qres{j}")
                    nc.scalar.copy(qr[:], ps[:, h, :])
                    q_res.append(qr)
                    nc.vector.tensor_copy(q8[:, j, :], ps[:, h, :])

        return [lambda: half(0, 4), lambda: half(4, DT)]

    def qpair(j0, wa, wb):
        for f in qpair_halves(j0, wa, wb):
            f()

    def kproj(j, w):
        ps = pp.tile([P, 2, SQ], F32, tag="ps2", name="ps2")
        for k in range(0, DT, 2):
            xt, kk = x8(k)
            nc.tensor.matmul(ps[:, 0, :], w[:, k:k + 2, :],
                             xt[:, kk:kk + 2, 0:SQ],
                             start=(k == 0), stop=(k == DT - 2), perf_mode=DR)
            nc.tensor.matmul(ps[:, 1, :], w[:, k:k + 2, :],
                             xt[:, kk:kk + 2, SQ:S],
                             start=(k == 0), stop=(k == DT - 2), perf_mode=DR)
        kj = acts.tile([P, S], FP8, tag=f"k8{j}", name=f"k8{j}")
        nc.vector.tensor_scalar_mul(
            kj[:].rearrange("p (c q) -> p c q", c=2), ps[:], 1.0 / ALPHA)
        k8[j] = kj

    def v_pair_halves(c, st0, wv):
        """Two emission closures, one per key tile; one psum tile + copy."""
        box = {}

        def half(si):
            if si == 0:
                box["ps"] = pp.tile([P, 2, SQ], F32, tag="ps2", name="ps2")
            ps = box["ps"]
            for k in range(0, DT, 2):
                xt, kk = x8(k)
                nc.tensor.matmul(
                    ps[:, si, :],
                    xt[:, kk:kk + 2, (st0 + si) * P:(st0 + si + 1) * P],
                    wv[:, k:k + 2, :],
                    start=(k == 0), stop=(k == DT - 2), perf_mode=DR)
            if si == 1:
                nc.vector.tensor_scalar_mul(
                    vr8[:, st0:st0 + 2, c * 8:(c + 1) * 8, 0:DH],
                    ps[:].rearrange("p s (h d) -> p s h d", d=DH),
                    1.0 / ALPHA)

        return [lambda: half(0), lambda: half(1)]

    def scx(j, st2):
        """scores + exp for head pair (2j, 2j+1), key tiles 2*st2, 2*st2+1."""
        e2 = e2p.tile([P, 2, 2, SQ], FP8, tag="e2", name="e2")
        for sti in range(2):
            st = 2 * st2 + sti
            sl = slice(st * P, (st + 1) * P)
            ps = pss.tile([P, 2, SQ], F32, tag="pss", name="pss")
            nc.tensor.matmul(ps[:, 0, :], k8[j][0:DH, sl], q8[0:DH, j, :],
                             start=True, stop=True)
            nc.tensor.matmul(ps[:, 1, :], k8[j][DH:P, sl], q8[DH:P, j, :],
                             start=True, stop=True)
            nc.scalar.activation(e2[:, sti, :, :], ps[:], AF.Exp, scale=SCALE)
        e2s[(j, st2)] = e2

    def pv_acc(j, st2):
        if st2 == 0:
            pvs[j] = (pvp.tile([DH + 1, SQ], F32, tag="pv", name="pv"),
                      pvp.tile([DH + 1, SQ], F32, tag="pv", name="pv"))
        pv0, pv1 = pvs[j]
        e2 = e2s.pop((j, st2))
        nc.tensor.matmul(pv0[:], vr8[:, 2 * st2:2 * st2 + 2, 2 * j, :],
                         e2[:, :, 0, :], start=(st2 == 0),
                         stop=(st2 == ST // 2 - 1), perf_mode=DR)
        nc.tensor.matmul(pv1[:], vr8[:, 2 * st2:2 * st2 + 2, 2 * j + 1, :],
                         e2[:, :, 1, :], start=(st2 == 0),
                         stop=(st2 == ST // 2 - 1), perf_mode=DR)

    def pv_all(j):
        for st2 in range(4):
            pv_acc(j, st2)

    def norm(j):
        pv0, pv1 = pvs.pop(j)
        for half, pv in ((0, pv0), (1, pv1)):
            rows = slice(half * DH, half * DH + DH)
            zh = sc2.tile([1, SQ], F32, tag="zh", name="zh")
            nc.vector.tensor_copy(zh[:], pv[DH:DH + 1, :])
            iz = sc2.tile([1, SQ], F32, tag="zh", name="iz")
            nc.vector.reciprocal_approx_fast(iz[:], zh[:])
            bz = sc2.tile([DH, SQ], F32, tag="sb", name="sb")
            nc.gpsimd.partition_broadcast(bz[:], iz[:])
            nc.vector.tensor_mul(attn8[rows, j, :], pv[0:DH, :], bz[:])

    # ---- block schedule: K(j) first so exp starts ~9us in; Q pairs and V
    # groups fill the PE between the exp-gated scores/PV chains. V(c=0)
    # completes by block 2 (pv pairs 0-3 read heads 0-7), V(c=1) by block 5.
    # attn*V for pair p runs 1-2 blocks after its exps; e2 pool depth 8
    # covers the two in-flight pairs. ----
    wv0 = wload("wv", wv_ap[:, 0], [P, DT, SQ], FP8, nsplit=2)
    kproj(0, wk0)
    qpair(0, wqa, wqb)
    scx(0, 0)
    vh = v_pair_halves(0, 0, wv0)
    vh[0]()
    scx(0, 1)
    vh[1]()
    scx(0, 2)
    scx(0, 3)
    wv1 = None
    preload = {}
    for j in range(1, DT):
        w = wload("wka" if j % 2 == 0 else "wkb", wk_ap[:, j], [P, DT, P],
                  FP8, nsplit=1)
        if j == 3:
            wv1 = wload("wv", wv_ap[:, 1], [P, DT, SQ], FP8, nsplit=2)
        # filler work items (~0.9-1.8us each) to slot between the exp-gated
        # scores; c0 V pairs done by block 2, c1 by block 5
        items = []
        vg = {1: [(0, 2), (0, 4)], 2: [(0, 6)], 3: [(1, 0)], 4: [(1, 2)],
              5: [(1, 4), (1, 6)]}.get(j, [])
        for c, st in vg:
            items += v_pair_halves(c, st, wv0 if c == 0 else wv1)
        if j < 4:
            items += qpair_halves(
                2 * j,
                wload("wqa", wq_ap[:, 2 * j], [P, DT, P], BF16, nsplit=1),
                wload("wqb", wq_ap[:, 2 * j + 1], [P, DT, P], BF16,
                      nsplit=1))
        if j >= 2:
            items.append(lambda jj=j - 2: (pv_all(jj), norm(jj)))
        if j == 6:
            # prefetch the first Wo / W1 stationaries during late attention
            items.append(lambda: preload.update(
                woa=wload("woa", wo_ap[:, 0], [P, DT, P], FP8, nsplit=1),
                wob=wload("wob", wo_ap[:, 1], [P, DT, P], FP8, nsplit=1)))
        if j == 7:
            items.append(lambda: preload.update(
                w1a=wload("w1a", w1_ap[:, 0], [P, DT, P], BF16),
                w1b=wload("w1b", w1_ap[:, 1], [P, DT, P], BF16)))
        kproj(j, w)
        for s in range(4):
            scx(j, s)
            take = max(1, (len(items) + 3 - s) // (4 - s))
            for it in items[:take]:
                it()
            items = items[take:]
        for it in items:
            it()
    for j in range(DT - 2, DT):
        pv_all(j)
        norm(j)

    # ---- output projection (fp8 DoubleRow) + relu + residual + LN1 stats ----
    h1, sq1 = [], []
    h18 = acts.tile([P, DT, SQ], BF16, tag="h18", name="h18")
    ps_sum1 = pvp.tile([1, SQ], F32, tag="pv", name="ps_sum1")
    ps_sq1 = pvp.tile([1, SQ], F32, tag="pv", name="ps_sq1")

    def colsum(ps_sum, ps_sq, src, sq, j):
        nc.tensor.matmul(ps_sum[:], ones_r[:], src[:],
                         start=(j == 0), stop=(j == DT - 1))
        nc.tensor.matmul(ps_sq[:], ones_b[:], sq[:],
                         start=(j == 0), stop=(j == DT - 1))

    for j0 in range(0, DT, 2):
        wa = preload["woa"] if j0 == 0 else wload(
            "woa", wo_ap[:, j0], [P, DT, P], FP8, nsplit=1)
        wb = preload["wob"] if j0 == 0 else wload(
            "wob", wo_ap[:, j0 + 1], [P, DT, P], FP8, nsplit=1)
        ps = pss.tile([P, 2, SQ], F32, tag="pss", name="ps_wo")
        for k in range(0, DT, 2):
            nc.tensor.matmul(ps[:, 0, :], wa[:, k:k + 2, :],
                             attn8[:, k:k + 2, :],
                             start=(k == 0), stop=(k == DT - 2), perf_mode=DR)
            nc.tensor.matmul(ps[:, 1, :], wb[:, k:k + 2, :],
                             attn8[:, k:k + 2, :],
                             start=(k == 0), stop=(k == DT - 2), perf_mode=DR)
        # lag-one interleave of the LN1 column sums
        for j in (j0 - 2, j0 - 1):
            if j >= 0:
                colsum(ps_sum1, ps_sq1, h1[j], sq1[j], j)
        for h in range(2):
            j = j0 + h
            rel = sc2.tile([P, SQ], F32R, tag="u", name="rel")
            nc.scalar.activation(rel[:], ps[:, h, :], AF.Relu,
                                 bias=consts[:, _C_BO + j:_C_BO + j + 1],
                                 scale=1.0 / ALPHA)
            t = acts.tile([P, SQ], F32R, tag=f"h1{j}", name=f"h1_{j}")
            nc.vector.tensor_add(t[:], rel[:], q_res[j][:])
            h1.append(t)
            nc.scalar.copy(h18[:, j, :], t[:])
            sq = acts.tile([P, SQ], BF16, tag=f"sq1{j}", name=f"sq1_{j}")
            nc.vector.tensor_mul(sq[:], t[:], t[:])
            sq1.append(sq)
    for j in (DT - 2, DT - 1):
        colsum(ps_sum1, ps_sq1, h1[j], sq1[j], j)

    # LN1 chain (gamma/beta folded into W1 / rank-1 W2 fold)
    s_sb = sc1.tile([1, SQ], F32, tag="s0", name="s_sb")
    nc.vector.tensor_copy(s_sb[:], ps_sum1[:])
    m2 = sc1.tile([1, SQ], F32, tag="s1", name="m2")
    nc.vector.tensor_mul(m2[:], s_sb[:], s_sb[:])
    a_t = sc1.tile([1, SQ], F32, tag="s2", name="a_t")
    nc.vector.scalar_tensor_tensor(a_t[:], m2[:], 1.0 / D, ps_sq1[:],
                                   op0=mybir.AluOpType.mult,
                                   op1=mybir.AluOpType.subtract)
    eps_t = sc1.tile([1, 1], F32, tag="eps", name="eps")
    nc.vector.memset(eps_t[:], EPS)
    sd1 = sc1.tile([1, SQ], F32, tag="s1", name="sd1")
    nc.scalar.activation(sd1[:], a_t[:], AF.Sqrt, bias=eps_t[:], scale=-1.0 / D)
    rstd1 = sc1.tile([1, SQ], F32, tag="s2", name="rstd1")
    nc.vector.reciprocal_approx_fast(rstd1[:], sd1[:])
    bneg1 = sc1.tile([1, SQ], F32, tag="s3", name="bneg1")
    nc.vector.scalar_tensor_tensor(bneg1[:], s_sb[:], -1.0 / D, rstd1[:],
                                   op0=mybir.AluOpType.mult,
                                   op1=mybir.AluOpType.mult)
    negmu_r = sc1.tile([1, SQ], BF16, tag="s4", name="negmu_r")
    nc.vector.tensor_scalar_mul(negmu_r[:], s_sb[:], -1.0 / D)
    sd_r = sc1.tile([1, SQ], BF16, tag="s5", name="sd_r")
    nc.vector.tensor_copy(sd_r[:], sd1[:])
    abc_sb = sc2.tile([P, SQ], F32, tag="sb", name="abc_sb")
    nc.gpsimd.partition_broadcast(abc_sb[:], rstd1[:])
    bbc_sb = sc2.tile([P, SQ], F32, tag="zh", name="bbc_sb")
    nc.gpsimd.partition_broadcast(bbc_sb[:], bneg1[:])

    # ---- FFN1 (bf16) ----
    hid_a = acts.tile([P, DT, SQ], BF16, tag="xtb_hi", name="hid_a")
    hid_b = acts.tile([P, DT, SQ], BF16, tag="hid_b", name="hid_b")
    for f0 in range(0, FT, 2):
        wa = preload["w1a"] if f0 == 0 else wload(
            "w1a", w1_ap[:, f0], [P, DT, P], BF16)
        wb = preload["w1b"] if f0 == 0 else wload(
            "w1b", w1_ap[:, f0 + 1], [P, DT, P], BF16)
        ps = pss.tile([P, 2, SQ], F32, tag="pss", name="ps_f1")
        for k in range(DT):
            nc.tensor.matmul(ps[:, 0, :], wa[:, k, :], h18[:, k, :],
                             start=(k == 0), stop=(k == DT - 1))
            nc.tensor.matmul(ps[:, 1, :], wb[:, k, :], h18[:, k, :],
                             start=(k == 0), stop=(k == DT - 1))
        hid = hid_a if f0 < DT else hid_b
        nc.scalar.copy(hid[:, f0 % DT:f0 % DT + 2, :], ps[:])

    # real ln1 for the residual (overlaps FFN1)
    ln1 = []
    for j in range(DT):
        u = sc2.tile([P, SQ], F32, tag="u", name="u")
        nc.vector.tensor_mul(u[:], h1[j][:], abc_sb[:])
        nc.vector.tensor_add(u[:], u[:], bbc_sb[:])
        d = acts.tile([P, SQ], F32R, tag=f"ln1{j}", name=f"ln1_{j}")
        nc.scalar.activation(d[:], u[:], AF.Identity,
                             bias=consts[:, _C_BE1 + j:_C_BE1 + j + 1],
                             scale=consts[:, _C_G1 + j:_C_G1 + j + 1])
        ln1.append(d)

    # ---- FFN2 (bf16) + rank-1 LN1 fold + relu + residual + LN2 stats ----
    f2, sq2 = [], []
    ps_sum2 = pvp.tile([1, SQ], F32, tag="pv", name="ps_sum2")
    ps_sq2 = pvp.tile([1, SQ], F32, tag="pv", name="ps_sq2")
    for j in range(DT):
        w = wload("w2", w2_ap[:, j], [P, FT, P], BF16, nsplit=4)
        ps = pss.tile([P, 2, SQ], F32, tag="pss", name="ps_f2")
        for f in range(FT):
            hid = hid_a if f < DT else hid_b
            nc.tensor.matmul(ps[:, 0, :], w[:, f, :], hid[:, f % DT, :],
                             start=(f == 0), stop=False)
        nc.tensor.matmul(ps[:, 0, :], fold[0:1, j * P:(j + 1) * P],
                         negmu_r[:], start=False, stop=False)
        nc.tensor.matmul(ps[:, 0, :], fold[0:1, D + j * P:D + (j + 1) * P],
                         sd_r[:], start=False, stop=True)
        if j > 0:
            colsum(ps_sum2, ps_sq2, f2[j - 1], sq2[j - 1], j - 1)
        rel = sc2.tile([P, SQ], F32R, tag="u", name="rel2")
        nc.vector.scalar_tensor_tensor(rel[:], ps[:, 0, :], 0.0, abc_sb[:],
                                       op0=mybir.AluOpType.max,
                                       op1=mybir.AluOpType.mult)
        t = acts.tile([P, SQ], F32R, tag=f"h1{j}", name=f"f2_{j}")
        nc.vector.tensor_add(t[:], rel[:], ln1[j][:])
        f2.append(t)
        sq = acts.tile([P, SQ], BF16, tag=f"sq1{j}", name=f"sq2_{j}")
        nc.scalar.activation(sq[:], t[:], AF.Square)
        sq2.append(sq)
    colsum(ps_sum2, ps_sq2, f2[DT - 1], sq2[DT - 1], DT - 1)

    # ---- LN2 chain + normalize (writes bf16, DMA per tile) ----
    s_sb2 = sc1.tile([1, SQ], F32, tag="s0", name="s_sb2")
    nc.vector.tensor_copy(s_sb2[:], ps_sum2[:])
    m22 = sc1.tile([1, SQ], F32, tag="s1", name="m22")
    nc.vector.tensor_mul(m22[:], s_sb2[:], s_sb2[:])
    a_t2 = sc1.tile([1, SQ], F32, tag="s2", name="a_t2")
    nc.vector.scalar_tensor_tensor(a_t2[:], m22[:], 1.0 / D, ps_sq2[:],
                                   op0=mybir.AluOpType.mult,
                                   op1=mybir.AluOpType.subtract)
    sd2 = sc1.tile([1, SQ], F32, tag="s1", name="sd2")
    nc.scalar.activation(sd2[:], a_t2[:], AF.Sqrt, bias=eps_t[:],
                         scale=-1.0 / D)
    rstd2 = sc1.tile([1, SQ], F32, tag="s2", name="rstd2")
    nc.vector.reciprocal_approx_fast(rstd2[:], sd2[:])
    bneg2 = sc1.tile([1, SQ], F32R, tag="s3", name="bneg2")
    nc.vector.scalar_tensor_tensor(bneg2[:], s_sb2[:], -1.0 / D, rstd2[:],
                                   op0=mybir.AluOpType.mult,
                                   op1=mybir.AluOpType.mult)
    a_r = sc1.tile([1, SQ], F32R, tag="s0", name="a_r2")
    nc.vector.tensor_copy(a_r[:], rstd2[:])
    ab = pss.tile([P, 2, SQ], F32, tag="pss", name="ab")
    nc.tensor.matmul(ab[:, 0, :], ones_row[:], a_r[:], start=True, stop=True)
    nc.tensor.matmul(ab[:, 1, :], ones_row[:], bneg2[:], start=True, stop=True)
    for j in range(DT):
        u = sc2.tile([P, SQ], F32, tag="u", name="u")
        nc.vector.tensor_mul(u[:], f2[j][:], ab[:, 0, :])
        nc.vector.tensor_add(u[:], u[:], ab[:, 1, :])
        d = acts.tile([P, SQ], BF16, tag=f"qres{j}", name=f"y_{j}")
        nc.scalar.activation(d[:], u[:], AF.Identity,
                             bias=consts[:, _C_BE2 + j:_C_BE2 + j + 1],
                             scale=consts[:, _C_G2 + j:_C_G2 + j + 1])
        nc.sync.dma_start(yT_ap[j * P:(j + 1) * P, :], d[:])


def build():
    nc = bacc.Bacc("TRN2", target_bir_lowering=False, debug=False,
                   num_devices=NCORES)
    xT_ap = nc.dram_tensor("xT", [P, DT, S], BF16, kind="ExternalInput").ap()
    x8_ap = nc.dram_tensor("xT8", [P, DT, S], FP8, kind="ExternalInput").ap()
    wq_ap = nc.dram_tensor("Wq", [P, DT, DT, P], BF16, kind="ExternalInput").ap()
    wk_ap = nc.dram_tensor("Wk", [P, DT, DT, P], FP8, kind="ExternalInput").ap()
    wv_ap = nc.dram_tensor("Wv", [P, 2, DT, SQ], FP8, kind="ExternalInput").ap()
    wo_ap = nc.dram_tensor("Wo", [P, DT, DT, P], FP8, kind="ExternalInput").ap()
    w1_ap = nc.dram_tensor("W1", [P, FT, DT, P], BF16, kind="ExternalInput").ap()
    w2_ap = nc.dram_tensor("W2", [P, DT, FT, P], BF16, kind="ExternalInput").ap()
    consts_ap = nc.dram_tensor("consts", [P, 64], F32, kind="ExternalInput").ap()
    ones_ap = nc.dram_tensor("ones", [P, 1], F32R, kind="ExternalInput").ap()
    onesrow_ap = nc.dram_tensor("ones_row", [1, P], F32R, kind="ExternalInput").ap()
    fold_ap = nc.dram_tensor("fold", [1, 2 * D], FP8, kind="ExternalInput").ap()
    yT_ap = nc.dram_tensor("yT", [D, SQ], BF16, kind="ExternalOutput").ap()
    aps = (xT_ap, x8_ap, wq_ap, wk_ap, wv_ap, wo_ap, w1_ap, w2_ap,
           consts_ap, ones_ap, onesrow_ap, fold_ap, yT_ap)
    from contextlib import ExitStack
    with tile.TileContext(nc) as tc, ExitStack() as ctx:
        _emit(ctx, tc, aps)
    nc.compile()
    return nc


_cached_nc = None


def _get_nc():
    global _cached_nc
    if _cached_nc is None:
        _cached_nc = build()
    return _cached_nc


def _to_bf16(a):
    return np.ascontiguousarray(np.asarray(a, np.float32)).astype(
        ml_dtypes.bfloat16)


def _to_fp8(a, scale):
    return np.clip(np.asarray(a, np.float32) * scale, -240.0, 240.0).astype(
        ml_dtypes.float8_e4m3)


def _prep_in_maps(x, Wq, Wk, Wv, Wo, bo, ln1_g, ln1_b, W1, b1, W2, b2,
                  ln2_g, ln2_b):
    f = np.float32
    consts = np.zeros((P, 64), f)
    consts[:, _C_BO:_C_BO + 8] = np.asarray(bo, f).reshape(8, P).T
    consts[:, _C_B1:_C_B1 + 16] = np.asarray(b1, f).reshape(16, P).T
    consts[:, _C_B2:_C_B2 + 8] = np.asarray(b2, f).reshape(8, P).T
    consts[:, _C_G1:_C_G1 + 8] = np.asarray(ln1_g, f).reshape(8, P).T
    consts[:, _C_BE1:_C_BE1 + 8] = np.asarray(ln1_b, f).reshape(8, P).T
    consts[:, _C_G2:_C_G2 + 8] = np.asarray(ln2_g, f).reshape(8, P).T
    consts[:, _C_BE2:_C_BE2 + 8] = np.asarray(ln2_b, f).reshape(8, P).T
    ones = np.ones((P, 1), f)
    ones_row = np.ones((1, P), f)
    W1f = np.asarray(W1, np.float64)
    W2f = np.asarray(W2, np.float64)
    g1v = np.asarray(ln1_g, np.float64)
    b1v = np.asarray(ln1_b, np.float64)
    c1 = np.asarray(b1, np.float64) + (b1v[:, None] * W1f).sum(axis=0)
    W1g = (g1v[:, None] * W1f).astype(f)
    w2g1 = (g1v[:, None] * W1f).sum(axis=0) @ W2f
    c2 = np.asarray(b2, np.float64) + c1 @ W2f
    fold = np.concatenate([w2g1, c2]).astype(f)[None, :]

    def pack_st(W, dtype_fn):
        # [D_in, N] -> [P, N/P, D_in/P, P] stationary tiles
        din, n = W.shape
        return np.ascontiguousarray(
            dtype_fn(np.asarray(W, f).reshape(din // P, P, n // P, P)
                     .transpose(1, 2, 0, 3)))

    shared = {
        "Wq": pack_st(np.asarray(Wq, f), _to_bf16),
        "Wk": pack_st(np.asarray(Wk, f), lambda a: _to_fp8(a, ALPHA)),
        "Wo": pack_st(np.asarray(Wo, f), lambda a: _to_fp8(a, ALPHA)),
        "W1": pack_st(W1g, _to_bf16),
        "W2": pack_st(np.asarray(W2, f), _to_bf16),
        "Wv": np.ascontiguousarray(
            _to_fp8(np.asarray(Wv, f).reshape(DT, P, 2, SQ)
                    .transpose(1, 2, 0, 3), ALPHA)),
        "consts": consts, "ones": ones, "ones_row": ones_row,
        "fold": _to_fp8(fold, 1.0),
    }
    xt = np.asarray(x, f).transpose(0, 2, 1)  # [B, D, S]
    in_maps = []
    for core in range(NCORES):
        b, off = core // 2, (core % 2) * SQ
        if off == 0:
            xrot = xt[b]
        else:
            xrot = np.concatenate([xt[b][:, off:], xt[b][:, :off]], axis=1)
        xr = xrot.reshape(DT, P, S).transpose(1, 0, 2)
        in_maps.append(dict(shared, xT=np.ascontiguousarray(_to_bf16(xr)),
                            xT8=np.ascontiguousarray(_to_fp8(xr, 1.0))))
    return in_maps


def run(inputs, trace=False, tmpdir=None):
    """Run the kernel on 8 cores. Returns (y, BassKernelResults)."""
    nc = _get_nc()
    in_maps = _prep_in_maps(
        inputs["x"], inputs["Wq"], inputs["Wk"], inputs["Wv"], inputs["Wo"],
        inputs["bo"], inputs["ln1_g"], inputs["ln1_b"], inputs["W1"],
        inputs["b1"], inputs["W2"], inputs["b2"], inputs["ln2_g"],
        inputs["ln2_b"])
    try:
        res = bass_utils.run_bass_kernel_spmd(nc, in_maps, list(range(NCORES)),
                                              trace=trace, tmpdir=tmpdir)
    except Exception:
        import time as _time
        _time.sleep(2.0)
        res = bass_utils.run_bass_kernel_spmd(nc, in_maps, list(range(NCORES)),
                                              trace=trace, tmpdir=tmpdir)
    y = np.empty((B, S, D), np.float32)
    for core in range(NCORES):
        b, off = core // 2, (core % 2) * SQ
        y[b, off:off + SQ, :] = res.results[core]["yT"].astype(np.float32).T
    return y, res


def kernel(x, mask, Wq, Wk, Wv, Wo, bo, ln1_g, ln1_b, W1, b1, W2, b2,
           ln2_g, ln2_b):
    # mask is all-ones per the problem spec -> identity in the reference.
    y, _ = run(dict(x=x, Wq=Wq, Wk=Wk, Wv=Wv, Wo=Wo, bo=bo, ln1_g=ln1_g,
                    ln1_b=ln1_b, W1=W1, b1=b1, W2=W2, b2=b2, ln2_g=ln2_g,
                    ln2_b=ln2_b))
    return y


# revision 30
# speedup vs baseline: 1.0374x; 1.0095x over previous
"""Trainium2 Bass kernel for nn_EncoderLayer (B=4, S=1024, D=1024, H=16, FF=2048).

Sharding: 8 cores = 4 batches x 2 sequence-halves, each core redundantly
computes K/V for its whole batch and runs the layer for its own 512 query
rows. Odd cores see the sequence rotated by 512 (softmax over keys is
permutation-invariant).

Mixed-precision PE pipeline:
  - K/V projections, QK^T scores, exp, attn*V and the output projection run
    in fp8e4 (TRN E4M3) with MatmulPerfMode.DoubleRow (two 128-row
    contraction tiles per instruction = 2x PE throughput). fp8 weights are
    pre-scaled by 32 on the host; the 1/32 is folded into PSUM->SBUF copies.
  - Q projection and both FFN matmuls stay bf16: their error lands on the
    residual stream where fp8 blows the 2e-2 budget (measured 5e-2 all-fp8
    vs 3.2e-3 with these three in bf16).

Schedule: the softmax exp stream (~57us on the scalar engine) is the
second-longest pole after the PE, so scores for head-pair j are emitted
immediately after the K projection of tile j, with attn*V lagging one pair
(PV(j-1) fills the PE while the vector engine re-quantizes k8[j]). V(c=0)
is interleaved into the j=0 block, V(c=1) into the j=4 block (attn*V for
pairs 0-3 only reads heads 0-7 = the c=0 half of V).

Layernorm plumbing (feature-major activations, stats via ones-column
matmuls, gamma/beta folded into W1 + rank-1 W2 corrections) as in v1; the
LN column-sum matmuls are interleaved (lag-one) into the Wo/FFN2 loops and
the LN2 normalize reads the A/B broadcast PSUM directly and writes bf16.
"""

import sys
import types

import numpy as np
import ml_dtypes


def _shim_axon_hooks():
    try:
        import antenv.axon_hooks  # noqa: F401
    except Exception:
        mod = types.ModuleType("antenv.axon_hooks")
        mod.get_axon_ntff_profile_hook = lambda: None
        mod.set_axon_ntff_profile_hook = lambda h: None
        sys.modules["antenv.axon_hooks"] = mod


_shim_axon_hooks()

from concourse import bacc, mybir, tile  # noqa: E402
from concourse import bass_utils  # noqa: E402

F32 = mybir.dt.float32
F32R = mybir.dt.float32r
BF16 = mybir.dt.bfloat16
FP8 = mybir.dt.float8e4
AF = mybir.ActivationFunctionType
DR = mybir.MatmulPerfMode.DoubleRow

B, S, D, H, DH, FF = 4, 1024, 1024, 16, 64, 2048
SQ = 512
P = 128
DT = D // P
FT = FF // P
ST = S // P
NCORES = 8
EPS = 1e-6
SCALE = 1.0 / 32.0
ALPHA = 32.0

_C_BO = 0
_C_B1 = 8
_C_B2 = 24
_C_G1 = 32
_C_BE1 = 40
_C_G2 = 48
_C_BE2 = 56


def _emit(ctx, tc, aps):
    nc = tc.nc
    (xT_ap, x8_ap, wq_ap, wk_ap, wv_ap, wo_ap, w1_ap, w2_ap, consts_ap,
     ones_ap, onesrow_ap, fold_ap, yT_ap) = aps

    acts = ctx.enter_context(tc.tile_pool(name="acts", bufs=1))
    wts = ctx.enter_context(tc.tile_pool(name="wts", bufs=2))
    e2p = ctx.enter_context(tc.tile_pool(name="e2p", bufs=7))
    sc2 = ctx.enter_context(tc.tile_pool(name="sc2", bufs=2))
    sc1 = ctx.enter_context(tc.tile_pool(name="sc1", bufs=1))
    pp = ctx.enter_context(tc.tile_pool(name="pp", bufs=1, space="PSUM"))
    pss = ctx.enter_context(tc.tile_pool(name="pss", bufs=2, space="PSUM"))
    pvp = ctx.enter_context(tc.tile_pool(name="pvp", bufs=2, space="PSUM"))

    def wload(tag, src_ap, shape, dtype, nsplit=2):
        w = wts.tile(shape, dtype, tag=tag, name=tag)
        step = shape[1] // nsplit
        for q in range(nsplit):
            nc.sync.dma_start(w[:, q * step:(q + 1) * step, :],
                              src_ap[:, q * step:(q + 1) * step, :])
        return w

    # ---- input DMA: xt8 + wk0 first (K0 gates the exp stream), then Q's ----
    xt8 = acts.tile([P, DT, S], FP8, tag="xt8", name="xt8")
    for q in range(DT):
        nc.sync.dma_start(xt8[:, q:q + 1, :], x8_ap[:, q:q + 1, :])

    def x8(k):
        return (xt8, k)
    wk0 = wload("wka", wk_ap[:, 0], [P, DT, P], FP8)
    wqa = wload("wqa", wq_ap[:, 0], [P, DT, P], BF16)
    wqb = wload("wqb", wq_ap[:, 1], [P, DT, P], BF16)
    xtb_a = acts.tile([P, 4, SQ], BF16, tag="xtb_a", name="xtb_a")
    nc.sync.dma_start(xtb_a[:, 0:2, :], xT_ap[:, 0:2, 0:SQ])
    nc.sync.dma_start(xtb_a[:, 2:4, :], xT_ap[:, 2:4, 0:SQ])
    xtb_b = acts.tile([P, 4, SQ], BF16, tag="xtb_b", name="xtb_b")
    nc.sync.dma_start(xtb_b[:, 0:2, :], xT_ap[:, 4:6, 0:SQ])
    nc.sync.dma_start(xtb_b[:, 2:4, :], xT_ap[:, 6:8, 0:SQ])
    consts = acts.tile([P, 64], F32, tag="consts", name="consts")
    nc.sync.dma_start(consts[:], consts_ap[:])
    ones_r = acts.tile([P, 1], F32R, tag="ones", name="ones")
    nc.sync.dma_start(ones_r[:], ones_ap[:])
    ones_row = acts.tile([1, P], F32R, tag="ones_row", name="ones_row")
    nc.sync.dma_start(ones_row[:], onesrow_ap[:])
    ones_b = acts.tile([P, 1], BF16, tag="ones_b", name="ones_b")
    nc.vector.memset(ones_b[:], 1.0)
    fold = acts.tile([1, 2 * D], FP8, tag="fold", name="fold")
    nc.sync.dma_start(fold[:], fold_ap[:])

    def xlo(k):
        return xtb_a[:, k, :] if k < 4 else xtb_b[:, k - 4, :]

    # ---- attention machinery ----
    q_res = []
    q8 = acts.tile([P, DT, SQ], FP8, tag="q8", name="q8")
    vr8 = acts.tile([P, ST, H, DH + 1], FP8, tag="vr8", name="vr8")
    nc.vector.memset(vr8[:, :, :, DH:DH + 1], 1.0)
    attn8 = acts.tile([P, DT, SQ], FP8, tag="attn8", name="attn8")
    k8 = [None] * DT
    e2s = {}
    pvs = {}

    def qpair_halves(j0, wa, wb):
        """Returns two emission closures (k 0:4 and k 4:8 + copies)."""
        box = {}

        def half(lo, hi):
            if lo == 0:
                box["ps"] = pp.tile([P, 2, SQ], F32, tag="ps2", name="ps2")
            ps = box["ps"]
            for k in range(lo, hi):
                nc.tensor.matmul(ps[:, 0, :], wa[:, k, :], xlo(k),
                                 start=(k == 0), stop=(k == DT - 1))
                nc.tensor.matmul(ps[:, 1, :], wb[:, k, :], xlo(k),
                                 start=(k == 0), stop=(k == DT - 1))
            if hi == DT:
                for h in range(2):
                    j = j0 + h
                    qr = acts.tile([P, SQ], F32R, tag=f"qres{j}",
                                   name=f"qres{j}")
                    nc.scalar.copy(qr[:], ps[:, h, :])
                    q_res.append(qr)
                    nc.vector.tensor_copy(q8[:, j, :], ps[:, h, :])

        return [lambda: half(0, 4), lambda: half(4, DT)]

    def qpair(j0, wa, wb):
        for f in qpair_halves(j0, wa, wb):
            f()

    def kproj(j, w):
        ps = pp.tile([P, 2, SQ], F32, tag="ps2", name="ps2")
        for k in range(0, DT, 2):
            xt, kk = x8(k)
            nc.tensor.matmul(ps[:, 0, :], w[:, k:k + 2, :],
                             xt[:, kk:kk + 2, 0:SQ],
                             start=(k == 0), stop=(k == DT - 2), perf_mode=DR)
            nc.tensor.matmul(ps[:, 1, :], w[:, k:k + 2, :],
                             xt[:, kk:kk + 2, SQ:S],
                             start=(k == 0), stop=(k == DT - 2), perf_mode=DR)
        kj = acts.tile([P, S], FP8, tag=f"k8{j}", name=f"k8{j}")
        nc.vector.tensor_scalar_mul(
            kj[:].rearrange("p (c q) -> p c q", c=2), ps[:], 1.0 / ALPHA)
        k8[j] = kj

    def v_pair_halves(c, st0, wv):
        """Two emission closures, one per key tile; one psum tile + copy."""
        box = {}

        def half(si):
            if si == 0:
                box["ps"] = pp.tile([P, 2, SQ], F32, tag="ps2", name="ps2")
            ps = box["ps"]
            for k in range(0, DT, 2):
                xt, kk = x8(k)
                nc.tensor.matmul(
                    ps[:, si, :],
                    xt[:, kk:kk + 2, (st0 + si) * P:(st0 + si + 1) * P],
                    wv[:, k:k + 2, :],
                    start=(k == 0), stop=(k == DT - 2), perf_mode=DR)
            if si == 1:
                nc.vector.tensor_scalar_mul(
                    vr8[:, st0:st0 + 2, c * 8:(c + 1) * 8, 0:DH],
                    ps[:].rearrange("p s (h d) -> p s h d", d=DH),
                    1.0 / ALPHA)

        return [lambda: half(0), lambda: half(1)]

    def scx(j, st2):
        """scores + exp for head pair (2j, 2j+1), key tiles 2*st2, 2*st2+1."""
        e2 = e2p.tile([P, 2, 2, SQ], FP8, tag="e2", name="e2")
        for sti in range(2):
            st = 2 * st2 + sti
            sl = slice(st * P, (st + 1) * P)
            ps = pss.tile([P, 2, SQ], F32, tag="pss", name="pss")
            nc.tensor.matmul(ps[:, 0, :], k8[j][0:DH, sl], q8[0:DH, j, :],
                             start=True, stop=True)
            nc.tensor.matmul(ps[:, 1, :], k8[j][DH:P, sl], q8[DH:P, j, :],
                             start=True, stop=True)
            nc.scalar.activation(e2[:, sti, :, :], ps[:], AF.Exp, scale=SCALE)
        e2s[(j, st2)] = e2

    def pv_acc(j, st2):
        if st2 == 0:
            pvs[j] = (pvp.tile([DH + 1, SQ], F32, tag="pv", name="pv"),
                      pvp.tile([DH + 1, SQ], F32, tag="pv", name="pv"))
        pv0, pv1 = pvs[j]
        e2 = e2s.pop((j, st2))
        nc.tensor.matmul(pv0[:], vr8[:, 2 * st2:2 * st2 + 2, 2 * j, :],
                         e2[:, :, 0, :], start=(st2 == 0),
                         stop=(st2 == ST // 2 - 1), perf_mode=DR)
        nc.tensor.matmul(pv1[:], vr8[:, 2 * st2:2 * st2 + 2, 2 * j + 1, :],
                         e2[:, :, 1, :], start=(st2 == 0),
                         stop=(st2 == ST // 2 - 1), perf_mode=DR)

    def pv_all(j):
        for st2 in range(4):
            pv_acc(j, st2)

    def norm(j):
        pv0, pv1 = pvs.pop(j)
        for half, pv in ((0, pv0), (1, pv1)):
            rows = slice(half * DH, half * DH + DH)
            zh = sc2.tile([1, SQ], F32, tag="zh", name="zh")
            nc.vector.tensor_copy(zh[:], pv[DH:DH + 1, :])
            iz = sc2.tile([1, SQ], F32, tag="zh", name="iz")
            nc.vector.reciprocal_approx_fast(iz[:], zh[:])
            bz = sc2.tile([DH, SQ], F32, tag="sb", name="sb")
            nc.gpsimd.partition_broadcast(bz[:], iz[:])
            nc.vector.tensor_mul(attn8[rows, j, :], pv[0:DH, :], bz[:])

    # ---- block schedule: K(j) first so exp starts ~9us in; Q pairs and V
    # groups fill the PE between the exp-gated scores/PV chains. V(c=0)
    # completes by block 2 (pv pairs 0-3 read heads 0-7), V(c=1) by block 5.
    # attn*V for pair p runs 1-2 blocks after its exps; e2 pool depth 8
    # covers the two in-flight pairs. ----
    wv0 = wload("wv", wv_ap[:, 0], [P, DT, SQ], FP8, nsplit=2)
    def khoist(j):
        """kproj(j) emitted as a filler item of block j-1, so its k8 copy
        enters the vector queue ahead of the deferred norm and the scores
        of block j are never vector-lagged."""
        w = wload("wka" if j % 2 == 0 else "wkb", wk_ap[:, j], [P, DT, P],
                  FP8, nsplit=1)
        kproj(j, w)

    kproj(0, wk0)
    qpair(0, wqa, wqb)
    scx(0, 0)
    vh = v_pair_halves(0, 0, wv0)
    vh[0]()
    scx(0, 1)
    vh[1]()
    scx(0, 2)
    khoist(1)
    scx(0, 3)
    wv1 = None
    preload = {}
    for j in range(1, DT):
        if j == 3:
            wv1 = wload("wv", wv_ap[:, 1], [P, DT, SQ], FP8, nsplit=2)
        # filler work items (~0.9-1.8us each) to slot between the exp-gated
        # scores; c0 V pairs done by block 2, c1 by block 5
        items = []
        vg = {1: [(0, 2), (0, 4)], 2: [(0, 6)], 3: [(1, 0)], 4: [(1, 2)],
              5: [(1, 4), (1, 6)]}.get(j, [])
        for c, st in vg:
            items += v_pair_halves(c, st, wv0 if c == 0 else wv1)
        if j < 4:
            items += qpair_halves(
                2 * j,
                wload("wqa", wq_ap[:, 2 * j], [P, DT, P], BF16, nsplit=1),
                wload("wqb", wq_ap[:, 2 * j + 1], [P, DT, P], BF16,
                      nsplit=1))
        if j < DT - 1:
            items.append(lambda jj=j + 1: khoist(jj))
        if j >= 2:
            items.append(lambda jj=j - 2: (pv_all(jj), norm(jj)))
        if j == 6:
            # prefetch the first Wo / W1 stationaries during late attention
            items.append(lambda: preload.update(
                woa=wload("woa", wo_ap[:, 0], [P, DT, P], FP8, nsplit=1),
                wob=wload("wob", wo_ap[:, 1], [P, DT, P], FP8, nsplit=1)))
        if j == 7:
            items.append(lambda: preload.update(
                w1a=wload("w1a", w1_ap[:, 0], [P, DT, P], BF16),
                w1b=wload("w1b", w1_ap[:, 1], [P, DT, P], BF16)))
        for s in range(4):
            scx(j, s)
            take = max(1, (len(items) + 3 - s) // (4 - s))
            for it in items[:take]:
                it()
            items = items[take:]
        for it in items:
            it()
    for j in range(DT - 2, DT):
        pv_all(j)
        norm(j)

    # ---- output projection (fp8 DoubleRow) + relu + residual + LN1 stats ----
    h1, sq1 = [], []
    h18 = acts.tile([P, DT, SQ], BF16, tag="h18", name="h18")
    ps_sum1 = pvp.tile([1, SQ], F32, tag="pv", name="ps_sum1")
    ps_sq1 = pvp.tile([1, SQ], F32, tag="pv", name="ps_sq1")

    def colsum(ps_sum, ps_sq, src, sq, j):
        nc.tensor.matmul(ps_sum[:], ones_r[:], src[:],
                         start=(j == 0), stop=(j == DT - 1))
        nc.tensor.matmul(ps_sq[:], ones_b[:], sq[:],
                         start=(j == 0), stop=(j == DT - 1))

    for j0 in range(0, DT, 2):
        wa = preload["woa"] if j0 == 0 else wload(
            "woa", wo_ap[:, j0], [P, DT, P], FP8, nsplit=1)
        wb = preload["wob"] if j0 == 0 else wload(
            "wob", wo_ap[:, j0 + 1], [P, DT, P], FP8, nsplit=1)
        ps = pss.tile([P, 2, SQ], F32, tag="pss", name="ps_wo")
        for k in range(0, DT, 2):
            nc.tensor.matmul(ps[:, 0, :], wa[:, k:k + 2, :],
                             attn8[:, k:k + 2, :],
                             start=(k == 0), stop=(k == DT - 2), perf_mode=DR)
            nc.tensor.matmul(ps[:, 1, :], wb[:, k:k + 2, :],
                             attn8[:, k:k + 2, :],
                             start=(k == 0), stop=(k == DT - 2), perf_mode=DR)
        # lag-one interleave of the LN1 column sums
        for j in (j0 - 2, j0 - 1):
            if j >= 0:
                colsum(ps_sum1, ps_sq1, h1[j], sq1[j], j)
        for h in range(2):
            j = j0 + h
            rel = sc2.tile([P, SQ], F32R, tag="u", name="rel")
            nc.scalar.activation(rel[:], ps[:, h, :], AF.Relu,
                                 bias=consts[:, _C_BO + j:_C_BO + j + 1],
                                 scale=1.0 / ALPHA)
            t = acts.tile([P, SQ], F32R, tag=f"h1{j}", name=f"h1_{j}")
            nc.vector.tensor_add(t[:], rel[:], q_res[j][:])
            h1.append(t)
            nc.scalar.copy(h18[:, j, :], t[:])
            sq = acts.tile([P, SQ], BF16, tag=f"sq1{j}", name=f"sq1_{j}")
            nc.vector.tensor_mul(sq[:], t[:], t[:])
            sq1.append(sq)
    for j in (DT - 2, DT - 1):
        colsum(ps_sum1, ps_sq1, h1[j], sq1[j], j)

    # LN1 chain (gamma/beta folded into W1 / rank-1 W2 fold)
    s_sb = sc1.tile([1, SQ], F32, tag="s0", name="s_sb")
    nc.vector.tensor_copy(s_sb[:], ps_sum1[:])
    m2 = sc1.tile([1, SQ], F32, tag="s1", name="m2")
    nc.vector.tensor_mul(m2[:], s_sb[:], s_sb[:])
    a_t = sc1.tile([1, SQ], F32, tag="s2", name="a_t")
    nc.vector.scalar_tensor_tensor(a_t[:], m2[:], 1.0 / D, ps_sq1[:],
                                   op0=mybir.AluOpType.mult,
                                   op1=mybir.AluOpType.subtract)
    eps_t = sc1.tile([1, 1], F32, tag="eps", name="eps")
    nc.vector.memset(eps_t[:], EPS)
    sd1 = sc1.tile([1, SQ], F32, tag="s1", name="sd1")
    nc.scalar.activation(sd1[:], a_t[:], AF.Sqrt, bias=eps_t[:], scale=-1.0 / D)
    rstd1 = sc1.tile([1, SQ], F32, tag="s2", name="rstd1")
    nc.vector.reciprocal_approx_fast(rstd1[:], sd1[:])
    bneg1 = sc1.tile([1, SQ], F32, tag="s3", name="bneg1")
    nc.vector.scalar_tensor_tensor(bneg1[:], s_sb[:], -1.0 / D, rstd1[:],
                                   op0=mybir.AluOpType.mult,
                                   op1=mybir.AluOpType.mult)
    negmu_r = sc1.tile([1, SQ], BF16, tag="s4", name="negmu_r")
    nc.vector.tensor_scalar_mul(negmu_r[:], s_sb[:], -1.0 / D)
    sd_r = sc1.tile([1, SQ], BF16, tag="s5", name="sd_r")
    nc.vector.tensor_copy(sd_r[:], sd1[:])
    abc_sb = sc2.tile([P, SQ], F32, tag="sb", name="abc_sb")
    nc.gpsimd.partition_broadcast(abc_sb[:], rstd1[:])
    bbc_sb = sc2.tile([P, SQ], F32, tag="zh", name="bbc_sb")
    nc.gpsimd.partition_broadcast(bbc_sb[:], bneg1[:])

    # ---- FFN1 (bf16) ----
    hid_a = acts.tile([P, DT, SQ], BF16, tag="xtb_hi", name="hid_a")
    hid_b = acts.tile([P, DT, SQ], BF16, tag="hid_b", name="hid_b")
    for f0 in range(0, FT, 2):
        wa = preload["w1a"] if f0 == 0 else wload(
            "w1a", w1_ap[:, f0], [P, DT, P], BF16)
        wb = preload["w1b"] if f0 == 0 else wload(
            "w1b", w1_ap[:, f0 + 1], [P, DT, P], BF16)
        ps = pss.tile([P, 2, SQ], F32, tag="pss", name="ps_f1")
        for k in range(DT):
            nc.tensor.matmul(ps[:, 0, :], wa[:, k, :], h18[:, k, :],
                             start=(k == 0), stop=(k == DT - 1))
            nc.tensor.matmul(ps[:, 1, :], wb[:, k, :], h18[:, k, :],
                             start=(k == 0), stop=(k == DT - 1))
        hid = hid_a if f0 < DT else hid_b
        nc.scalar.copy(hid[:, f0 % DT:f0 % DT + 2, :], ps[:])

    # real ln1 for the residual (overlaps FFN1)
    ln1 = []
    for j in range(DT):
        u = sc2.tile([P, SQ], F32, tag="u", name="u")
        nc.vector.tensor_mul(u[:], h1[j][:], abc_sb[:])
        nc.vector.tensor_add(u[:], u[:], bbc_sb[:])
        d = acts.tile([P, SQ], F32R, tag=f"ln1{j}", name=f"ln1_{j}")
        nc.scalar.activation(d[:], u[:], AF.Identity,
                             bias=consts[:, _C_BE1 + j:_C_BE1 + j + 1],
                             scale=consts[:, _C_G1 + j:_C_G1 + j + 1])
        ln1.append(d)

    # ---- FFN2 (bf16) + rank-1 LN1 fold + relu + residual + LN2 stats ----
    f2, sq2 = [], []
    ps_sum2 = pvp.tile([1, SQ], F32, tag="pv", name="ps_sum2")
    ps_sq2 = pvp.tile([1, SQ], F32, tag="pv", name="ps_sq2")
    for j in range(DT):
        w = wload("w2", w2_ap[:, j], [P, FT, P], BF16, nsplit=4)
        ps = pss.tile([P, 2, SQ], F32, tag="pss", name="ps_f2")
        for f in range(FT):
            hid = hid_a if f < DT else hid_b
            nc.tensor.matmul(ps[:, 0, :], w[:, f, :], hid[:, f % DT, :],
                             start=(f == 0), stop=False)
        nc.tensor.matmul(ps[:, 0, :], fold[0:1, j * P:(j + 1) * P],
                         negmu_r[:], start=False, stop=False)
        nc.tensor.matmul(ps[:, 0, :], fold[0:1, D + j * P:D + (j + 1) * P],
                         sd_r[:], start=False, stop=True)
        if j > 0:
            colsum(ps_sum2, ps_sq2, f2[j - 1], sq2[j - 1], j - 1)
        rel = sc2.tile([P, SQ], F32R, tag="u", name="rel2")
        nc.vector.scalar_tensor_tensor(rel[:], ps[:, 0, :], 0.0, abc_sb[:],
                                       op0=mybir.AluOpType.max,
                                       op1=mybir.AluOpType.mult)
        t = acts.tile([P, SQ], F32R, tag=f"h1{j}", name=f"f2_{j}")
        nc.vector.tensor_add(t[:], rel[:], ln1[j][:])
        f2.append(t)
        sq = acts.tile([P, SQ], BF16, tag=f"sq1{j}", name=f"sq2_{j}")
        nc.scalar.activation(sq[:], t[:], AF.Square)
        sq2.append(sq)
    colsum(ps_sum2, ps_sq2, f2[DT - 1], sq2[DT - 1], DT - 1)

    # ---- LN2 chain + normalize (writes bf16, DMA per tile) ----
    s_sb2 = sc1.tile([1, SQ], F32, tag="s0", name="s_sb2")
    nc.vector.tensor_copy(s_sb2[:], ps_sum2[:])
    m22 = sc1.tile([1, SQ], F32, tag="s1", name="m22")
    nc.vector.tensor_mul(m22[:], s_sb2[:], s_sb2[:])
    a_t2 = sc1.tile([1, SQ], F32, tag="s2", name="a_t2")
    nc.vector.scalar_tensor_tensor(a_t2[:], m22[:], 1.0 / D, ps_sq2[:],
                                   op0=mybir.AluOpType.mult,
                                   op1=mybir.AluOpType.subtract)
    sd2 = sc1.tile([1, SQ], F32, tag="s1", name="sd2")
    nc.scalar.activation(sd2[:], a_t2[:], AF.Sqrt, bias=eps_t[:],
                         scale=-1.0 / D)
    rstd2 = sc1.tile([1, SQ], F32, tag="s2", name="rstd2")
    nc.vector.reciprocal_approx_fast(rstd2[:], sd2[:])
    bneg2 = sc1.tile([1, SQ], F32R, tag="s3", name="bneg2")
    nc.vector.scalar_tensor_tensor(bneg2[:], s_sb2[:], -1.0 / D, rstd2[:],
                                   op0=mybir.AluOpType.mult,
                                   op1=mybir.AluOpType.mult)
    a_r = sc1.tile([1, SQ], F32R, tag="s0", name="a_r2")
    nc.vector.tensor_copy(a_r[:], rstd2[:])
    ab = pss.tile([P, 2, SQ], F32, tag="pss", name="ab")
    nc.tensor.matmul(ab[:, 0, :], ones_row[:], a_r[:], start=True, stop=True)
    nc.tensor.matmul(ab[:, 1, :], ones_row[:], bneg2[:], start=True, stop=True)
    for j in range(DT):
        u = sc2.tile([P, SQ], F32, tag="u", name="u")
        nc.vector.tensor_mul(u[:], f2[j][:], ab[:, 0, :])
        nc.vector.tensor_add(u[:], u[:], ab[:, 1, :])
        d = acts.tile([P, SQ], BF16, tag=f"qres{j}", name=f"y_{j}")
        nc.scalar.activation(d[:], u[:], AF.Identity,
                             bias=consts[:, _C_BE2 + j:_C_BE2 + j + 1],
                             scale=consts[:, _C_G2 + j:_C_G2 + j + 1])
        nc.sync.dma_start(yT_ap[j * P:(j + 1) * P, :], d[:])


def build():
    nc = bacc.Bacc("TRN2", target_bir_lowering=False, debug=False,
                   num_devices=NCORES)
    xT_ap = nc.dram_tensor("xT", [P, DT, S], BF16, kind="ExternalInput").ap()
    x8_ap = nc.dram_tensor("xT8", [P, DT, S], FP8, kind="ExternalInput").ap()
    wq_ap = nc.dram_tensor("Wq", [P, DT, DT, P], BF16, kind="ExternalInput").ap()
    wk_ap = nc.dram_tensor("Wk", [P, DT, DT, P], FP8, kind="ExternalInput").ap()
    wv_ap = nc.dram_tensor("Wv", [P, 2, DT, SQ], FP8, kind="ExternalInput").ap()
    wo_ap = nc.dram_tensor("Wo", [P, DT, DT, P], FP8, kind="ExternalInput").ap()
    w1_ap = nc.dram_tensor("W1", [P, FT, DT, P], BF16, kind="ExternalInput").ap()
    w2_ap = nc.dram_tensor("W2", [P, DT, FT, P], BF16, kind="ExternalInput").ap()
    consts_ap = nc.dram_tensor("consts", [P, 64], F32, kind="ExternalInput").ap()
    ones_ap = nc.dram_tensor("ones", [P, 1], F32R, kind="ExternalInput").ap()
    onesrow_ap = nc.dram_tensor("ones_row", [1, P], F32R, kind="ExternalInput").ap()
    fold_ap = nc.dram_tensor("fold", [1, 2 * D], FP8, kind="ExternalInput").ap()
    yT_ap = nc.dram_tensor("yT", [D, SQ], BF16, kind="ExternalOutput").ap()
    aps = (xT_ap, x8_ap, wq_ap, wk_ap, wv_ap, wo_ap, w1_ap, w2_ap,
           consts_ap, ones_ap, onesrow_ap, fold_ap, yT_ap)
    from contextlib import ExitStack
    with tile.TileContext(nc) as tc, ExitStack() as ctx:
        _emit(ctx, tc, aps)
    nc.compile()
    return nc


_cached_nc = None


def _get_nc():
    global _cached_nc
    if _cached_nc is None:
        _cached_nc = build()
    return _cached_nc


def _to_bf16(a):
    return np.ascontiguousarray(np.asarray(a, np.float32)).astype(
        ml_dtypes.bfloat16)


def _to_fp8(a, scale):
    return np.clip(np.asarray(a, np.float32) * scale, -240.0, 240.0).astype(
        ml_dtypes.float8_e4m3)


def _prep_in_maps(x, Wq, Wk, Wv, Wo, bo, ln1_g, ln1_b, W1, b1, W2, b2,
                  ln2_g, ln2_b):
    f = np.float32
    consts = np.zeros((P, 64), f)
    consts[:, _C_BO:_C_BO + 8] = np.asarray(bo, f).reshape(8, P).T
    consts[:, _C_B1:_C_B1 + 16] = np.asarray(b1, f).reshape(16, P).T
    consts[:, _C_B2:_C_B2 + 8] = np.asarray(b2, f).reshape(8, P).T
    consts[:, _C_G1:_C_G1 + 8] = np.asarray(ln1_g, f).reshape(8, P).T
    consts[:, _C_BE1:_C_BE1 + 8] = np.asarray(ln1_b, f).reshape(8, P).T
    consts[:, _C_G2:_C_G2 + 8] = np.asarray(ln2_g, f).reshape(8, P).T
    consts[:, _C_BE2:_C_BE2 + 8] = np.asarray(ln2_b, f).reshape(8, P).T
    ones = np.ones((P, 1), f)
    ones_row = np.ones((1, P), f)
    W1f = np.asarray(W1, np.float64)
    W2f = np.asarray(W2, np.float64)
    g1v = np.asarray(ln1_g, np.float64)
    b1v = np.asarray(ln1_b, np.float64)
    c1 = np.asarray(b1, np.float64) + (b1v[:, None] * W1f).sum(axis=0)
    W1g = (g1v[:, None] * W1f).astype(f)
    w2g1 = (g1v[:, None] * W1f).sum(axis=0) @ W2f
    c2 = np.asarray(b2, np.float64) + c1 @ W2f
    fold = np.concatenate([w2g1, c2]).astype(f)[None, :]

    def pack_st(W, dtype_fn):
        # [D_in, N] -> [P, N/P, D_in/P, P] stationary tiles
        din, n = W.shape
        return np.ascontiguousarray(
            dtype_fn(np.asarray(W, f).reshape(din // P, P, n // P, P)
                     .transpose(1, 2, 0, 3)))

    shared = {
        "Wq": pack_st(np.asarray(Wq, f), _to_bf16),
        "Wk": pack_st(np.asarray(Wk, f), lambda a: _to_fp8(a, ALPHA)),
        "Wo": pack_st(np.asarray(Wo, f), lambda a: _to_fp8(a, ALPHA)),
        "W1": pack_st(W1g, _to_bf16),
        "W2": pack_st(np.asarray(W2, f), _to_bf16),
        "Wv": np.ascontiguousarray(
            _to_fp8(np.asarray(Wv, f).reshape(DT, P, 2, SQ)
                    .transpose(1, 2, 0, 3), ALPHA)),
        "consts": consts, "ones": ones, "ones_row": ones_row,
        "fold": _to_fp8(fold, 1.0),
    }
    xt = np.asarray(x, f).transpose(0, 2, 1)  # [B, D, S]
    in_maps = []
    for core in range(NCORES):
        b, off = core // 2, (core % 2) * SQ
        if off == 0:
            xrot = xt[b]
        else:
            xrot = np.concatenate([xt[b][:, off:], xt[b][:, :off]], axis=1)
        xr = xrot.reshape(DT, P, S).transpose(1, 0, 2)
        in_maps.append(dict(shared, xT=np.ascontiguousarray(_to_bf16(xr)),
                            xT8=np.ascontiguousarray(_to_fp8(xr, 1.0))))
    return in_maps


def run(inputs, trace=False, tmpdir=None):
    """Run the kernel on 8 cores. Returns (y, BassKernelResults)."""
    nc = _get_nc()
    in_maps = _prep_in_maps(
        inputs["x"], inputs["Wq"], inputs["Wk"], inputs["Wv"], inputs["Wo"],
        inputs["bo"], inputs["ln1_g"], inputs["ln1_b"], inputs["W1"],
        inputs["b1"], inputs["W2"], inputs["b2"], inputs["ln2_g"],
        inputs["ln2_b"])
    try:
        res = bass_utils.run_bass_kernel_spmd(nc, in_maps, list(range(NCORES)),
                                              trace=trace, tmpdir=tmpdir)
    except Exception:
        import time as _time
        _time.sleep(2.0)
        res = bass_utils.run_bass_kernel_spmd(nc, in_maps, list(range(NCORES)),
                                              trace=trace, tmpdir=tmpdir)
    y = np.empty((B, S, D), np.float32)
    for core in range(NCORES):
        b, off = core // 2, (core % 2) * SQ
        y[b, off:off + SQ, :] = res.results[core]["yT"].astype(np.float32).T
    return y, res


def kernel(x, mask, Wq, Wk, Wv, Wo, bo, ln1_g, ln1_b, W1, b1, W2, b2,
           ln2_g, ln2_b):
    # mask is all-ones per the problem spec -> identity in the reference.
    y, _ = run(dict(x=x, Wq=Wq, Wk=Wk, Wv=Wv, Wo=Wo, bo=bo, ln1_g=ln1_g,
                    ln1_b=ln1_b, W1=W1, b1=b1, W2=W2, b2=b2, ln2_g=ln2_g,
                    ln2_b=ln2_b))
    return y
